# revision 14
# baseline (speedup 1.0000x reference)
"""BiGCN (2-layer bidirectional GCN + global add pool) on 8 Trainium2 NeuronCores.

Strategy (hardcoded for the nn_BiGCN_graphcl problem shapes):
  - Nodes are sharded graph-aligned: core c owns graphs [128c, 128c+128) and
    their (contiguous, batch-sorted) node range, padded to a common NPC.
  - Layer-1 node features hn1 = dinv * (x @ W1) are computed REPLICATED: every
    core computes the full [R, 256] table locally from globally reordered,
    dinv-prescaled fp8 copies of x (one per direction), using fp8 DoubleRow
    matmuls (K=256 in one instruction).  NO collectives at all.  Table rows
    pack both directions: row = [hn_td | hn_bu] in fp8 (256 bytes), written
    chunk-major so the edge phase can start as soon as chunk 0 lands.
  - Self-loops are folded into the edge lists (an extra edge v->v per node),
    so the epilogue is just out = dinv * agg + b with no separate hn term.
  - Per direction (td / bu), edges are assigned to the core owning their
    target node.  Each core gathers 256B packed rows for its edge shard with
    dma_gather (4 SWDGE queues) and segment-sums on the TensorEngine into
    per-window (128-node) PSUM tiles (fp8 x fp8 matmuls).  Slots are sorted
    by target position within each (window, block) run, so each 128-slot
    group's targets span a narrow band: the one-hot (DVE is_equal against an
    iota constant, fp8 out) is built only band-wide (64 cols for ~98% of
    groups, 128 otherwise), halving DVE work.
  - The final output is graph-pooled, so layer 2 collapses algebraically:
        out[g] = (sum_s Mp[s,g] * h1[s]) @ W2 + n_g * b2
    with Mp host-precomputed in fp8 (x8 scaled; W2 carries the /8).  Each
    core contracts its LOCAL h1 rows against Mp into a [128f, 1024g] partial
    accumulator; the host sums the 8 per-core partial outputs.
  - The SPMD program is identical on all cores: all per-core variation lives
    in uploaded index/data tensors; run lengths are padded to the max across
    cores (pad slots gather row 0 of the block and carry dstloc=-1 so their
    one-hot column is zero).
"""

import math
import numpy as np
import ml_dtypes

BF16 = ml_dtypes.bfloat16
FP8 = ml_dtypes.float8_e4m3

# ---------------------------------------------------------------- problem cfg
FULL_CFG = dict(
    N=100000, E=1600000, IN_FEATS=256, HIDDEN=128, OUT_FEATS=128,
    NUM_GRAPHS=1024, N_CORES=8, SW=8, NBLK=4, PIECE_G=16, MP_SCALE=8.0,
)


def _round_up(x, m):
    return (x + m - 1) // m * m


# =====================================================================
# Host-side metadata construction
# =====================================================================

def build_partition(batch, cfg, deg_td=None, deg_bu=None):
    """Graph-aligned node partition. Returns dict with per-core node ranges.

    If degree arrays are given, each core's local node order is permuted so
    that per-window (128-node) degree sums cluster just under multiples of
    4*128 edges per (window, src-block) run, minimizing ceil-128 padding."""
    N, C, G = cfg["N"], cfg["N_CORES"], cfg["NUM_GRAPHS"]
    gpc = G // C  # graphs per core
    starts = np.searchsorted(batch, np.arange(0, G + 1, gpc))
    counts = np.diff(starts)
    NPC = max(128, _round_up(int(counts.max()), 128))
    W = NPC // 128
    node_core = np.searchsorted(starts[1:], np.arange(N), side="right")
    node_local = np.arange(N) - starts[node_core]

    if deg_td is not None:
        for c in range(C):
            lo, hi = starts[c], starts[c + 1]
            cnt = hi - lo
            dt = deg_td[lo:hi].astype(np.int64)
            db = deg_bu[lo:hi].astype(np.int64)
            order = np.argsort(-(dt + db), kind="stable")
            tg_t = np.full(W, dt.sum() / W)
            tg_b = np.full(W, db.sum() / W)
            rem_t = tg_t.astype(np.float64).copy()
            rem_b = tg_b.astype(np.float64).copy()
            room = np.full(W, 128, np.int64)
            assign = np.empty(cnt, np.int64)
            for j in order:
                score = np.minimum(rem_t - dt[j], rem_b - db[j])
                score[room <= 0] = -np.inf
                w = int(np.argmax(score))
                assign[j] = w
                rem_t[w] -= dt[j]
                rem_b[w] -= db[j]
                room[w] -= 1
            # positions: window-major order
            slot_in_w = np.zeros(W, np.int64)
            newloc = np.empty(cnt, np.int64)
            for j in range(cnt):
                w = assign[j]
                newloc[j] = w * 128 + slot_in_w[w]
                slot_in_w[w] += 1
            node_local[lo:hi] = newloc

    # ---- chunk decomposition: 4 window-chunks, sized so per-(window, chunk)
    # gather runs land just under multiples of 128, and each chunk's block of
    # 8*128*w_q table rows stays within int16 index range. ----
    NBLK = cfg["NBLK"]
    mean_w = max(1.0, (deg_td.sum() + deg_bu.sum()) / (2.0 * C * W)) if deg_td is not None else 128.0
    wmax = min(W, (32767 // (128 * C)))

    def padfrac(wb):
        r = wb / W * mean_w  # mean edges per (window, this-chunk) run
        if r <= 0:
            return 0.0
        margin = 1.6 * np.sqrt(r) + 6
        gslots = 128 * np.ceil((r + margin) / 128)
        return (gslots - r) * 1.0

    best = None
    for w1 in range(1, wmax + 1):
        for w2 in range(w1, wmax + 1):
            for w3 in range(w2, wmax + 1):
                w4 = W - w1 - w2 - w3
                if w4 < w3 or w4 > wmax:
                    continue
                cost = padfrac(w1) + padfrac(w2) + padfrac(w3) + padfrac(w4)
                if best is None or cost < best[0]:
                    best = (cost, (w1, w2, w3, w4))
    ws = sorted(best[1]) if best else [W]
    # small chunks first: their table writes complete earliest, letting
    # the gather phase start sooner
    cw = np.concatenate([[0], np.cumsum(ws)])
    assert cw[-1] == W

    chunk_of_w = np.searchsorted(cw[1:], np.arange(W), side="right")
    q = chunk_of_w[np.minimum(node_local // 128, W - 1)]
    rpr = 128 * np.diff(cw)  # rows per rank per chunk
    base = np.concatenate([[0], np.cumsum(rpr * C)])
    table_row = base[q] + node_core * rpr[q] + (node_local - 128 * cw[q])
    bounds = [int(b) for b in base]
    return dict(starts=starts, counts=counts, NPC=NPC, gpc=gpc,
                node_core=node_core.astype(np.int64),
                node_local=node_local.astype(np.int64),
                table_row=table_row.astype(np.int64),
                cw=cw, bounds=bounds)


def build_direction_meta(gather_nodes, target_nodes, part, cfg):
    """Build per-core gather index / dstloc arrays and the uniform group
    structure for one edge direction.  Self-loops v->v are appended to the
    edge list so the epilogue needs no separate self term.  Slots within each
    (super, block, window) run are sorted by target position so each group's
    targets span a narrow band; per group a band (b0, wid in {64,128}) is
    chosen uniformly across cores and dstloc is stored band-relative.
    """
    N, C = cfg["N"], cfg["N_CORES"]
    SW, NBLK = cfg["SW"], cfg["NBLK"]
    NPC = part["NPC"]
    W = NPC // 128
    NS = (W + SW - 1) // SW

    loop = np.arange(N, dtype=np.int64)
    gather_nodes = np.concatenate([np.asarray(gather_nodes), loop])
    target_nodes = np.concatenate([np.asarray(target_nodes), loop])

    deg = np.bincount(target_nodes, minlength=N).astype(np.float64)

    bounds = part["bounds"]
    assert len(bounds) == NBLK + 1
    assert all(bounds[i + 1] - bounds[i] <= 32767 for i in range(NBLK))
    bounds_arr = np.array(bounds[1:-1])

    tr_g = part["table_row"][gather_nodes]
    t_core = part["node_core"][target_nodes]
    t_local = part["node_local"][target_nodes]
    lw = t_local // 128          # window
    dloc = t_local % 128         # position within window
    blk = np.searchsorted(bounds_arr, tr_g, side="right")
    idxv = tr_g - np.array(bounds[:-1])[blk]
    sup = lw // SW

    # per (core, s, b, w) counts -> uniform G
    keyW = (sup * NBLK + blk) * W + lw  # key within a core
    nkeys = NS * NBLK * W
    counts = np.zeros((C, nkeys), np.int64)
    for c in range(C):
        m = t_core == c
        counts[c] = np.bincount(keyW[m], minlength=nkeys)
    max_counts = counts.max(axis=0).reshape(NS, NBLK, W)

    G = np.ceil(max_counts / 128).astype(np.int64)  # groups per (s,b,w)
    # ensure every window has at least one group (psum must be written)
    for s in range(NS):
        w_lo, w_hi = s * SW, min((s + 1) * SW, W)
        for w in range(w_lo, w_hi):
            if G[s, :, w].sum() == 0:
                G[s, 0, w] = 1
        G[s, :, :w_lo] = 0
        G[s, :, w_hi:] = 0
    Gflat = G.reshape(nkeys)
    gmax = int(Gflat.max())

    # ---- per-core sorted runs (sorted by dloc within each key run) ----
    percore = []
    lo_all = np.full((nkeys, gmax), 128, np.int64)
    hi_all = np.full((nkeys, gmax), -1, np.int64)
    for c in range(C):
        m = t_core == c
        k = keyW[m]
        order = np.lexsort((dloc[m], k))
        ks = k[order]
        dvs = dloc[m][order]
        ivs = idxv[m][order]
        run_start = np.searchsorted(ks, np.arange(nkeys))
        rank = np.arange(len(ks)) - run_start[ks]
        percore.append((ks, dvs, ivs, rank))
        grp = np.minimum(rank // 128, gmax - 1)
        np.minimum.at(lo_all, (ks, grp), dvs)
        np.maximum.at(hi_all, (ks, grp), dvs)

    # 64-wide band, base in {0,64} (PSUM partition-base constraint: base 0/64
    # for 64-wide writes; base 32 allows only 32-wide)
    b0c = np.where(lo_all >= 64, 64, 0)
    fits = (hi_all < 0) | (hi_all <= b0c + 63)
    wid_all = np.where(fits, 64, 128)
    b0_all = np.where(fits & (hi_all >= 0), b0c, 0)

    # structure: per (s,b): window col bases, totals, group band records
    struct = []
    for s in range(NS):
        w_lo, w_hi = s * SW, min((s + 1) * SW, W)
        for b in range(NBLK):
            g_list = G[s, b, w_lo:w_hi]
            base = np.concatenate([[0], np.cumsum(g_list)])
            struct.append(dict(s=s, b=b, w_lo=w_lo, w_hi=w_hi,
                               g_list=g_list, g_base=base,
                               G=int(g_list.sum())))
    # global group/column offsets + per-group band records
    offG = 0
    off16 = 0
    n64 = n128 = 0
    for sb in struct:
        sb["offG"] = offG
        sb["off16"] = off16
        groups = []
        s, b = sb["s"], sb["b"]
        for i, w in enumerate(range(sb["w_lo"], sb["w_hi"])):
            key = (sb["s"] * NBLK + b) * W + w
            for g in range(int(sb["g_list"][i])):
                wid = int(wid_all[key, g])
                b0 = int(b0_all[key, g])
                if wid == 64:
                    groups.append((w, 64, n64, b0))
                    n64 += 1
                else:
                    groups.append((w, 128, n128, b0))
                    n128 += 1
        sb["groups"] = groups
        offG += sb["G"]
        off16 += sb["G"] * 8  # 128 slots / 16
    CG = offG
    CG64, CG128 = n64, n128
    Gmax = max((sb["G"] for sb in struct), default=1)

    # per-super class offsets (for dl tile loads)
    supers = []
    for si in range(NS):
        sbs = struct[si * NBLK:(si + 1) * NBLK]
        g64 = [g for sb in sbs for g in sb["groups"] if g[1] == 64]
        g128 = [g for sb in sbs for g in sb["groups"] if g[1] == 128]
        supers.append(dict(
            off64=g64[0][2] if g64 else 0, cnt64=len(g64),
            off128=g128[0][2] if g128 else 0, cnt128=len(g128)))

    # ---- per-edge slot assignment (per core) ----
    idx_all = np.zeros((C, 128, CG * 8), np.int16)
    dl64_all = np.full((C, 128, max(CG64, 1)), -1.0, BF16)
    dl128_all = np.full((C, 128, max(CG128, 1)), -1.0, BF16)
    # per (key, g): global slot base, class, class idx, b0
    slot_base = np.zeros((nkeys, gmax), np.int64)
    cls_arr = np.zeros((nkeys, gmax), np.int64)
    cidx_arr = np.zeros((nkeys, gmax), np.int64)
    for sb in struct:
        s, b = sb["s"], sb["b"]
        gi = 0
        for i, w in enumerate(range(sb["w_lo"], sb["w_hi"])):
            key = (s * NBLK + b) * W + w
            for g in range(int(sb["g_list"][i])):
                gslot = (sb["offG"] + sb["g_base"][i] + g) * 128
                slot_base[key, g] = gslot
                _, cls, cidx, _ = sb["groups"][gi]
                cls_arr[key, g] = cls
                cidx_arr[key, g] = cidx
                gi += 1

    for c in range(C):
        ks, dvs, ivs, rank = percore[c]
        grp = np.minimum(rank // 128, gmax - 1)
        slot = slot_base[ks, grp] + (rank - grp * 128)
        dshift = dvs - b0_all[ks, grp]
        # idx wrapped layout: slot j -> (j%16, j//16), replicated x8
        prow = slot % 16
        pcol = slot // 16
        tmp = np.zeros((16, CG * 8), np.int16)
        tmp[prow, pcol] = ivs.astype(np.int16)
        idx_all[c] = np.tile(tmp, (8, 1))
        cls_s = cls_arr[ks, grp]
        cidx_s = cidx_arr[ks, grp]
        srow = slot % 128
        m64 = cls_s == 64
        dl64_all[c, srow[m64], cidx_s[m64]] = dshift[m64].astype(BF16)
        m128 = ~m64
        dl128_all[c, srow[m128], cidx_s[m128]] = dshift[m128].astype(BF16)

    return dict(deg=deg, struct=struct, supers=supers, CG=CG,
                CG64=max(CG64, 1), CG128=max(CG128, 1), Gmax=Gmax, NS=NS, W=W,
                bounds=bounds, idx_all=idx_all,
                dl64_all=dl64_all, dl128_all=dl128_all)


def build_Mp(src, dst, batch, part, td_deg, bu_deg, cfg):
    """Folded layer-2 coefficients, local-row form, fp8 with MP_SCALE folded.

    out_graph[g] = (sum_s Mp[s,g] * h1[s]) @ W2 + n_g * b2, with the sum over
    LOCAL nodes s of each core (rows in node_local order)."""
    C, N, G = cfg["N_CORES"], cfg["N"], cfg["NUM_GRAPHS"]
    NPC = part["NPC"]
    nc_ = part["node_core"]
    nl = part["node_local"]
    batch = np.asarray(batch)

    dinv_td = (1.0 / np.sqrt(td_deg)).astype(np.float32)
    dinv_bu = (1.0 / np.sqrt(bu_deg)).astype(np.float32)

    M_td = np.zeros((C, NPC, G), np.float32)
    M_bu = np.zeros((C, NPC, G), np.float32)
    # td: value row src, target dst -> coeff dinv_td[dst] at (core(src), loc(src), g(dst))
    np.add.at(M_td, (nc_[src], nl[src], batch[dst]), dinv_td[dst])
    # bu: value row dst, target src -> coeff dinv_bu[src]
    np.add.at(M_bu, (nc_[dst], nl[dst], batch[src]), dinv_bu[src])
    # self-loop diagonals
    allv = np.arange(N)
    np.add.at(M_td, (nc_[allv], nl[allv], batch[allv]), dinv_td[allv])
    np.add.at(M_bu, (nc_[allv], nl[allv], batch[allv]), dinv_bu[allv])
    # fold the value-side dinv (from hn2 = dinv * (h1@W2)) into M
    f_td = np.zeros((C, NPC), np.float32)
    f_bu = np.zeros((C, NPC), np.float32)
    f_td[nc_[allv], nl[allv]] = dinv_td[allv]
    f_bu[nc_[allv], nl[allv]] = dinv_bu[allv]
    M_td *= f_td[:, :, None] * cfg["MP_SCALE"]
    M_bu *= f_bu[:, :, None] * cfg["MP_SCALE"]
    n_g = np.bincount(batch, minlength=G).astype(np.float32)
    return dict(Mp_td=M_td.astype(FP8), Mp_bu=M_bu.astype(FP8), n_g=n_g,
                dinv_td=dinv_td, dinv_bu=dinv_bu)


def build_all_inputs(x, edge_index, batch, Ws, bs, cfg):
    """Produce per-core in_maps plus structural metadata."""
    C = cfg["N_CORES"]
    N = cfg["N"]
    src = np.asarray(edge_index[0])
    dst = np.asarray(edge_index[1])
    part = build_partition(batch, cfg,
                           deg_td=np.bincount(dst, minlength=N),
                           deg_bu=np.bincount(src, minlength=N))
    NPC = part["NPC"]
    W = NPC // 128
    R = C * NPC
    RW = R // 128

    td = build_direction_meta(src, dst, part, cfg)   # gather src row, scatter to dst
    bu = build_direction_meta(dst, src, part, cfg)   # reversed
    mp = build_Mp(src, dst, batch, part, td["deg"], bu["deg"], cfg)

    PIECE_G = cfg["PIECE_G"]
    iota128 = np.tile(np.arange(128, dtype=np.float32), PIECE_G)[None, :] \
        .repeat(128, 0).astype(BF16)
    iota64 = np.tile(np.arange(64, dtype=np.float32), PIECE_G)[None, :] \
        .repeat(128, 0).astype(BF16)

    # global fp8 x in table-row order + global per-row dinv (both directions
    # interleaved) used to scale the A1 psum before the fp8 table store
    tr = part["table_row"]
    xTg = np.zeros((256, R), FP8)
    xTg[:, tr] = np.asarray(x).T.astype(FP8)
    dinvg = np.ones((R, 2), np.float32)
    dinvg[tr, 0] = mp["dinv_td"]
    dinvg[tr, 1] = mp["dinv_bu"]
    dinvg = np.ascontiguousarray(
        dinvg.reshape(RW, 128, 2).transpose(1, 0, 2).reshape(128, RW * 2))
    W1cat = np.concatenate([Ws[0], Ws[2]], axis=1).astype(FP8)  # [256, 256]

    ngb2 = np.concatenate([np.outer(mp["n_g"], bs[1]),
                           np.outer(mp["n_g"], bs[3])], axis=1).astype(np.float32)

    # per-core tensors
    in_maps = []
    for c in range(C):
        lo, hi = part["starts"][c], part["starts"][c + 1]
        li = part["node_local"][lo:hi]
        dloc = np.ones((128, W, 2), np.float32)
        dloc[li % 128, li // 128, 0] = mp["dinv_td"][lo:hi]
        dloc[li % 128, li // 128, 1] = mp["dinv_bu"][lo:hi]
        im = dict(
            xTg=xTg,
            dinvg=dinvg,
            W1cat=W1cat,
            dinv_loc=dloc.reshape(128, W * 2),
            iota64=iota64, iota128=iota128,
            Mp_td=mp["Mp_td"][c], Mp_bu=mp["Mp_bu"][c],
            idx_td=td["idx_all"][c], idx_bu=bu["idx_all"][c],
            dl64_td=td["dl64_all"][c], dl128_td=td["dl128_all"][c],
            dl64_bu=bu["dl64_all"][c], dl128_bu=bu["dl128_all"][c],
            W_td2=(Ws[1] / cfg["MP_SCALE"]).astype(BF16),
            W_bu2=(Ws[3] / cfg["MP_SCALE"]).astype(BF16),
            b_td1=np.tile(bs[0][None, :], (128, 1)).astype(np.float32),
            b_bu1=np.tile(bs[2][None, :], (128, 1)).astype(np.float32),
        )
        in_maps.append(im)
    meta = dict(part=part, td=td, bu=bu, NPC=NPC, W=W, cfg=cfg,
                R=R, RW=RW, ngb2=ngb2)
    return in_maps, meta


# =====================================================================
# Bass program
# =====================================================================

def build_bass(meta):
    import concourse.bacc as bacc
    import concourse.mybir as mybir
    import concourse.tile as tile

    cfg = meta["cfg"]
    C = cfg["N_CORES"]
    NPC, W = meta["NPC"], meta["W"]
    IN, HID = cfg["IN_FEATS"], cfg["HIDDEN"]
    NBLK = cfg["NBLK"]
    NG = cfg["NUM_GRAPHS"]
    PIECE_G = cfg["PIECE_G"]
    R, RW = meta["R"], meta["RW"]
    f32, bf16, i16 = mybir.dt.float32, mybir.dt.bfloat16, mybir.dt.int16
    fp8 = mybir.dt.float8e4
    DR = mybir.MatmulPerfMode.DoubleRow

    nc = bacc.Bacc("TRN2", target_bir_lowering=False, debug=False, num_devices=C,
                   num_swdge_queues=4)

    # ---- I/O ----
    ten = {}
    def inp(name, shape, dt):
        ten[name] = nc.dram_tensor(name, shape, dt, kind="ExternalInput")
        return ten[name]

    inp("dinv_loc", [128, W * 2], f32)
    inp("iota64", [128, PIECE_G * 64], bf16)
    inp("iota128", [128, PIECE_G * 128], bf16)
    inp("xTg", [IN, R], fp8)
    inp("dinvg", [128, RW * 2], f32)
    inp("W1cat", [IN, 2 * HID], fp8)
    for d in ("td", "bu"):
        m = meta[d]
        inp(f"idx_{d}", [128, m["CG"] * 8], i16)
        inp(f"dl64_{d}", [128, m["CG64"]], bf16)
        inp(f"dl128_{d}", [128, m["CG128"]], bf16)
        inp(f"Mp_{d}", [NPC, NG], fp8)
        inp(f"W_{d}2", [HID, HID], bf16)
        inp(f"b_{d}1", [128, HID], f32)
    out_t = nc.dram_tensor("out", [NG, 2 * HID], f32, kind="ExternalOutput")

    # internal DRAM: per-block packed hn tables [rows_b, 256] fp8
    bounds = meta["td"]["bounds"]
    table = {}
    for b in range(NBLK):
        table[b] = nc.dram_tensor(
            f"table{b}", [bounds[b + 1] - bounds[b], 2 * HID], fp8,
            kind="Internal")

    from contextlib import ExitStack
    with tile.TileContext(nc) as tc, ExitStack() as stack:
        def pool(name, bufs, space="SBUF"):
            return stack.enter_context(tc.tile_pool(name=name, bufs=bufs, space=space))

        const = pool("const", 1)
        xt_p = pool("xt", 6)
        hnb_p = pool("hnb", 3)
        idx_p = pool("idx", 6)
        dl_p = pool("dl", 6)
        gat_p = pool("gat", 20)              # gathered edge tiles (fp8)
        oh_p = pool("oh", 6)                 # one-hot tiles (fp8)
        mp_p = pool("mp", 3)                 # Mp super tiles
        win_p = pool("win", 4, "PSUM")       # window psum, 4 windows/bank
        pps_p = pool("pps", 3, "PSUM")       # A1 psum pairs / P-partial halves
        hps_p = pool("hps", 1, "PSUM")       # final projection psum
        epi_p = pool("epi", 6)               # epilogue sbuf tiles
        h1_p = pool("h1", 4)
        accs = pool("accs", 1)               # P accumulator (SBUF, f32)

        # ---- constants in SBUF ----
        iota_t = {}
        for wd, cols in ((64, PIECE_G * 64), (128, PIECE_G * 128)):
            t = const.tile([128, cols], bf16, tag=f"iota{wd}")
            nc.sync.dma_start(t[:], ten[f"iota{wd}"][:])
            iota_t[wd] = t
        W1c = []
        for kk in range(IN // 128):
            t = const.tile([128, 2 * HID], fp8, tag=f"W1c{kk}", name=f"W1c{kk}")
            nc.sync.dma_start(t[:], ten["W1cat"][kk * 128:(kk + 1) * 128, :])
            W1c.append(t)
        dinvg_t = const.tile([128, RW * 2], f32, tag="dinvg")
        nc.sync.dma_start(dinvg_t[:], ten["dinvg"][:])
        W2t, bt = {}, {}
        for d in ("td", "bu"):
            t = const.tile([128, HID], bf16, tag=f"W2_{d}", name=f"W2_{d}")
            nc.sync.dma_start(t[:], ten[f"W_{d}2"][:])
            W2t[d] = t
            t = const.tile([128, HID], f32, tag=f"b_{d}1", name=f"bt_{d}1")
            nc.sync.dma_start(t[:], ten[f"b_{d}1"][:])
            bt[d] = t
        dinvl_t = const.tile([128, W * 2], f32, tag="dinvl")
        nc.sync.dma_start(dinvl_t[:], ten["dinv_loc"][:])
        zrow = const.tile([1, 512], bf16, tag="zrow")
        nc.gpsimd.memset(zrow[:], 0.0)

        # P accumulator [128f, td 1024g | bu 1024g] f32
        acc = accs.tile([128, 2 * NG], f32, tag="acc", name="acc")
        nc.gpsimd.memset(acc[:], 0.0)

        cw = meta["part"]["cw"]
        chunk_base_oct = [int(cw[q]) for q in range(NBLK + 1)]  # x8 windows = octets

        # ---- A1 (replicated): hn = dinv * (x @ W1), both directions packed,
        # written chunk-major to the fp8 tables ----
        nK = IN // 128

        def emit_a1_octet(oct_i):
            q = 0
            while chunk_base_oct[q + 1] * 8 <= oct_i * 8:
                q += 1
            xts = []
            for kk in range(nK):
                t = xt_p.tile([128, 8 * 128], fp8, tag="xt", name=f"xa_{oct_i}_{kk}")
                nc.sync.dma_start(
                    t[:], ten["xTg"][kk * 128:(kk + 1) * 128,
                                     oct_i * 1024:(oct_i + 1) * 1024])
                xts.append(t)
            hnb = hnb_p.tile([128, 8, 2 * HID], fp8, tag="hnb", name=f"hnb_{oct_i}")
            for j0 in range(0, 8, 2):
                hps = pps_p.tile([128, 2, 2 * HID], f32, tag="pps")
                for j in (j0, j0 + 1):
                    for kk in range(nK):
                        nc.tensor.matmul(hps[:, j - j0, :],
                                         xts[kk][:, j * 128:(j + 1) * 128],
                                         W1c[kk][:], start=(kk == 0),
                                         stop=(kk == nK - 1),
                                         skip_group_check=True)
                c0 = (oct_i * 8 + j0) * 2
                nc.vector.tensor_tensor(
                    out=hnb[:, j0:j0 + 2, :].rearrange("p j (t f) -> p (j t) f", f=HID),
                    in0=hps[:].rearrange("p j (t f) -> p (j t) f", f=HID),
                    in1=dinvg_t[:, c0:c0 + 4]
                    .rearrange("p (x o) -> p x o", o=1)
                    .to_broadcast([128, 4, HID]),
                    op=mybir.AluOpType.mult)
            rowb = oct_i * 1024 - bounds[q]
            nc.scalar.dma_start(
                table[q][rowb:rowb + 1024, :].rearrange("(j p) f -> p j f", p=128),
                hnb[:])

        # ---- edge-phase per-super loads (idx/dl on sync pre-A1 for the first
        # supers, then on the gpsimd queue so they never sit behind A1 DMA) ----
        sup_tiles = {}

        def emit_super_loads(d, si, eng):
            m = meta[d]
            structs = m["struct"]
            sb0 = structs[si * NBLK]
            supG = sum(x["G"] for x in structs[si * NBLK:(si + 1) * NBLK])
            rec = m["supers"][si]
            ts = {}
            if supG > 0:
                t = idx_p.tile([128, supG * 8], i16, tag="idx")
                eng.dma_start(t[:], ten[f"idx_{d}"]
                              [:, sb0["off16"]:sb0["off16"] + supG * 8])
                ts["idx"] = t
            for wd in (64, 128):
                cnt = rec[f"cnt{wd}"]
                if cnt > 0:
                    t = dl_p.tile([128, cnt], bf16, tag=f"dl{wd}")
                    off = rec[f"off{wd}"]
                    eng.dma_start(t[:], ten[f"dl{wd}_{d}"][:, off:off + cnt])
                    ts[f"dl{wd}"] = t
            sup_tiles[(d, si)] = ts

        PREF = 3
        for si in range(min(PREF, meta["td"]["NS"])):
            for d in ("td", "bu"):
                emit_super_loads(d, si, nc.sync)

        for oct_i in range(RW // 8):
            emit_a1_octet(oct_i)

        # ---- edge phase (layer-1 aggregation + fused pooled layer-2) ----
        qn = [0]

        def epilogue(d, w, pt, mpt, wi, pps, w_lo, w_hi):
            di = 0 if d == "td" else 1
            o1 = epi_p.tile([128, HID], f32, tag="o1")
            nc.vector.scalar_tensor_tensor(
                out=o1[:], in0=pt[:], scalar=dinvl_t[:, w * 2 + di:w * 2 + di + 1],
                in1=bt[d][:],
                op0=mybir.AluOpType.mult, op1=mybir.AluOpType.add)
            h1 = h1_p.tile([128, HID], fp8, tag="h1")
            nc.scalar.activation(h1[:], o1[:], mybir.ActivationFunctionType.Relu)
            for h in range(2):
                nc.tensor.matmul(pps[h][:], h1[:], mpt[:, wi, h * 512:(h + 1) * 512],
                                 start=(w == w_lo), stop=(w == w_hi - 1),
                                 skip_group_check=True)

        def edge_phase(d):
            m = meta[d]
            doff = 0 if d == "td" else HID
            structs = m["struct"]
            supers = m["supers"]
            last_mm = {}
            for sbi, sb in enumerate(structs):
                gi = 0
                for i, w in enumerate(range(sb["w_lo"], sb["w_hi"])):
                    for g in range(int(sb["g_list"][i])):
                        if sb["g_list"][i] > 0:
                            last_mm[w] = (sbi, int(sb["g_base"][i]) + g)
            quad_tiles = {}
            def win_q(w):
                q = w // 4
                if q not in quad_tiles:
                    qt = win_p.tile([128, 512], f32, tag="win",
                                    name=f"win_{d}_{q}")
                    nc.tensor.matmul(qt[:], zrow[0:1, 0:128], zrow[0:1, 0:512],
                                     start=True, stop=False, skip_group_check=True)
                    quad_tiles[q] = qt
                return quad_tiles[q]
            it_sup = None
            dlt = {}
            sup_rec = None
            sup_off16 = 0
            for sbi, sb in enumerate(structs):
                si = sbi // NBLK
                if sbi % NBLK == 0:
                    sup_off16 = sb["off16"]
                    sup_rec = supers[si]
                    if (d, si) not in sup_tiles:
                        emit_super_loads(d, si, nc.gpsimd)
                    ts = sup_tiles[(d, si)]
                    it_sup = ts.get("idx")
                    dlt = {64: ts.get("dl64"), 128: ts.get("dl128")}
                    # Mp tile for the whole super (both layer-2 halves)
                    nsw = sb["w_hi"] - sb["w_lo"]
                    mpt = mp_p.tile([128, nsw, NG], fp8, tag="mp",
                                    name=f"mp_{d}_{si}")
                    nc.scalar.dma_start(
                        mpt[:], ten[f"Mp_{d}"][sb["w_lo"] * 128:sb["w_hi"] * 128, :]
                        .rearrange("(j p) g -> p j g", p=128))
                    sup_mpt = mpt
                G = sb["G"]
                if G == 0:
                    continue
                r16 = sb["off16"] - sup_off16
                # flat (window, group-record) list for this (s,b)
                wg = []
                gi = 0
                for i, w in enumerate(range(sb["w_lo"], sb["w_hi"])):
                    for g in range(int(sb["g_list"][i])):
                        wg.append((w, int(sb["g_base"][i]) + g) + sb["groups"][gi])
                        gi += 1
                # split into pieces of <= PIECE_G groups
                npiece = (G + PIECE_G - 1) // PIECE_G
                for pi in range(npiece):
                    g0 = pi * G // npiece
                    g1 = (pi + 1) * G // npiece
                    pg = g1 - g0
                    gt = gat_p.tile([128, pg, 2 * HID], fp8, tag="gat")
                    qn[0] += 1
                    nc.gpsimd.dma_gather(gt[:], table[sb["b"]][:],
                                         it_sup[:, r16 + g0 * 8:r16 + g1 * 8],
                                         num_idxs=pg * 128,
                                         num_idxs_reg=pg * 128, elem_size=2 * HID,
                                         single_packet=False, queue_num=qn[0] % 4)
                    # one-hots per band class
                    ohs = {}
                    cls_lo = {}
                    for wd in (64, 128):
                        recs = [r for r in wg[g0:g1] if r[3] == wd]
                        if not recs:
                            continue
                        c_lo = recs[0][4]
                        ncl = len(recs)
                        cls_lo[wd] = c_lo
                        oh = oh_p.tile([128, PIECE_G * wd], fp8, tag=f"oh{wd}")
                        ohs[wd] = oh
                        s_off = sup_rec[f"off{wd}"]
                        nc.vector.tensor_tensor(
                            out=oh[:, :ncl * wd].rearrange("p (g f) -> p g f", f=wd),
                            in0=dlt[wd][:, c_lo - s_off:c_lo - s_off + ncl]
                            .rearrange("p (g o) -> p g o", o=1)
                            .to_broadcast([128, ncl, wd]),
                            in1=iota_t[wd][:, :ncl * wd]
                            .rearrange("p (g f) -> p g f", f=wd),
                            op=mybir.AluOpType.is_equal)
                    for (w, g, _w2, wd, cidx, b0) in wg[g0:g1]:
                        qt = win_q(w)
                        ci = cidx - cls_lo[wd]
                        nc.tensor.matmul(
                            qt[b0:b0 + wd, (w % 4) * 128:(w % 4 + 1) * 128],
                            ohs[wd][:, ci * wd:(ci + 1) * wd],
                            gt[:, g - g0, doff:doff + HID],
                            start=False, stop=(last_mm[w] == (sbi, g)),
                            skip_group_check=True)
                # epilogues for completed supers: after last block of super
                if sb["b"] == NBLK - 1:
                    pps = [pps_p.tile([128, 512], f32, tag="pps",
                                      name=f"pps{h}_{d}_{sb['s']}")
                           for h in range(2)]
                    for wi, w in enumerate(range(sb["w_lo"], sb["w_hi"])):
                        pt = win_q(w)[:, (w % 4) * 128:(w % 4 + 1) * 128]
                        epilogue(d, w, pt, sup_mpt, wi, pps, sb["w_lo"], sb["w_hi"])
                    # fold the super's P-partial into the SBUF accumulator
                    aoff = 0 if d == "td" else NG
                    for h in range(2):
                        nc.vector.tensor_tensor(
                            out=acc[:, aoff + h * 512:aoff + (h + 1) * 512],
                            in0=acc[:, aoff + h * 512:aoff + (h + 1) * 512],
                            in1=pps[h][:], op=mybir.AluOpType.add)
                    quad_tiles.clear()
                    yield sb["w_hi"]
                else:
                    yield None

        def run_layer():
            gens = {"td": edge_phase("td"), "bu": edge_phase("bu")}
            done = {"td": False, "bu": False}
            while not all(done.values()):
                for d in ("td", "bu"):
                    if done[d]:
                        continue
                    try:
                        next(gens[d])
                    except StopIteration:
                        done[d] = True

        run_layer()

        # ---- final projection: out_partial[g] = P^T @ W2 (host sums partials) ----
        for d, (aoff, ooff) in (("td", (0, 0)), ("bu", (NG, HID))):
            for gc in range(NG // 128):
                pb = epi_p.tile([128, 128], bf16, tag="pb")
                nc.vector.tensor_copy(pb[:], acc[:, aoff + gc * 128:aoff + (gc + 1) * 128])
                fps = hps_p.tile([128, 4, HID], f32, tag="hps")
                nc.tensor.matmul(fps[:, 0, :], pb[:], W2t[d][:], start=True, stop=True,
                                 skip_group_check=True)
                ob = epi_p.tile([128, HID], f32, tag="ob")
                nc.vector.tensor_copy(ob[:], fps[:, 0, :])
                nc.sync.dma_start(out_t[gc * 128:(gc + 1) * 128, ooff:ooff + HID], ob[:])

    nc.compile()
    return nc


# =====================================================================
# Entry point
# =====================================================================

def _run(inputs, cfg, trace=False):
    from concourse import bass_utils
    x = np.asarray(inputs["x"], np.float32)
    edge_index = np.asarray(inputs["edge_index"])
    batch = np.asarray(inputs["batch"])
    Ws = [np.asarray(inputs[k], np.float32) for k in ("W_td1", "W_td2", "W_bu1", "W_bu2")]
    bs = [np.asarray(inputs[k], np.float32) for k in ("b_td1", "b_td2", "b_bu1", "b_bu2")]
    in_maps, meta = build_all_inputs(x, edge_index, batch, Ws, bs, cfg)
    nc = build_bass(meta)
    res = bass_utils.run_bass_kernel_spmd(
        nc, in_maps, core_ids=list(range(cfg["N_CORES"])), trace=trace)
    out = sum(res.results[c]["out"].astype(np.float64) for c in range(cfg["N_CORES"]))
    out = out + meta["ngb2"].astype(np.float64)
    return out.astype(np.float32), res


def kernel(**inputs):
    out, _ = _run(inputs, FULL_CFG, trace=False)
    return out


# revision 17
# speedup vs baseline: 1.2478x; 1.2478x over previous
"""BiGCN (2-layer bidirectional GCN + global add pool) on 8 Trainium2 NeuronCores.

Strategy (hardcoded for the nn_BiGCN_graphcl problem shapes):
  - Nodes are sharded graph-aligned: core c owns graphs [128c, 128c+128) and
    their (contiguous, batch-sorted) node range, padded to a common NPC.
  - Layer-1 node features hn1 = dinv * (x @ W1) are computed REPLICATED: every
    core computes the full [R, 256] table locally from globally reordered,
    dinv-prescaled fp8 copies of x (one per direction), using fp8 DoubleRow
    matmuls (K=256 in one instruction).  NO collectives at all.  Table rows
    pack both directions: row = [hn_td | hn_bu] in fp8 (256 bytes), written
    chunk-major so the edge phase can start as soon as chunk 0 lands.
  - Self-loops are folded into the edge lists (an extra edge v->v per node),
    so the epilogue is just out = dinv * agg + b with no separate hn term.
  - Per direction (td / bu), edges are assigned to the core owning their
    target node.  Each core gathers 256B packed rows for its edge shard with
    dma_gather (4 SWDGE queues) and segment-sums on the TensorEngine into
    per-window (128-node) PSUM tiles (fp8 x fp8 matmuls).  Slots are sorted
    by target position within each (window, block) run, so each 128-slot
    group's targets span a narrow band: the one-hot (DVE is_equal against an
    iota constant, fp8 out) is built only band-wide (64 cols for ~98% of
    groups, 128 otherwise), halving DVE work.
  - The final output is graph-pooled, so layer 2 collapses algebraically:
        out[g] = (sum_s Mp[s,g] * h1[s]) @ W2 + n_g * b2
    with Mp host-precomputed in fp8 (x8 scaled; W2 carries the /8).  Each
    core contracts its LOCAL h1 rows against Mp into a [128f, 1024g] partial
    accumulator; the host sums the 8 per-core partial outputs.
  - The SPMD program is identical on all cores: all per-core variation lives
    in uploaded index/data tensors; run lengths are padded to the max across
    cores (pad slots gather row 0 of the block and carry dstloc=-1 so their
    one-hot column is zero).
"""

import math
import numpy as np
import ml_dtypes

BF16 = ml_dtypes.bfloat16
FP8 = ml_dtypes.float8_e4m3

# ---------------------------------------------------------------- problem cfg
FULL_CFG = dict(
    N=100000, E=1600000, IN_FEATS=256, HIDDEN=128, OUT_FEATS=128,
    NUM_GRAPHS=1024, N_CORES=8, SW=8, NBLK=4, PIECE_G=32, MP_SCALE=8.0,
)


def _round_up(x, m):
    return (x + m - 1) // m * m


# =====================================================================
# Host-side metadata construction
# =====================================================================

def build_partition(batch, cfg, deg_td=None, deg_bu=None):
    """Graph-aligned node partition. Returns dict with per-core node ranges.

    If degree arrays are given, each core's local node order is permuted so
    that per-window (128-node) degree sums cluster just under multiples of
    4*128 edges per (window, src-block) run, minimizing ceil-128 padding."""
    N, C, G = cfg["N"], cfg["N_CORES"], cfg["NUM_GRAPHS"]
    gpc = G // C  # graphs per core
    starts = np.searchsorted(batch, np.arange(0, G + 1, gpc))
    counts = np.diff(starts)
    NPC = max(128, _round_up(int(counts.max()), 128))
    W = NPC // 128
    node_core = np.searchsorted(starts[1:], np.arange(N), side="right")
    node_local = np.arange(N) - starts[node_core]

    if deg_td is not None:
        for c in range(C):
            lo, hi = starts[c], starts[c + 1]
            cnt = hi - lo
            dt = deg_td[lo:hi].astype(np.int64)
            db = deg_bu[lo:hi].astype(np.int64)
            order = np.argsort(-(dt + db), kind="stable")
            tg_t = np.full(W, dt.sum() / W)
            tg_b = np.full(W, db.sum() / W)
            rem_t = tg_t.astype(np.float64).copy()
            rem_b = tg_b.astype(np.float64).copy()
            room = np.full(W, 128, np.int64)
            assign = np.empty(cnt, np.int64)
            for j in order:
                score = np.minimum(rem_t - dt[j], rem_b - db[j])
                score[room <= 0] = -np.inf
                w = int(np.argmax(score))
                assign[j] = w
                rem_t[w] -= dt[j]
                rem_b[w] -= db[j]
                room[w] -= 1
            # positions: window-major order
            slot_in_w = np.zeros(W, np.int64)
            newloc = np.empty(cnt, np.int64)
            for j in range(cnt):
                w = assign[j]
                newloc[j] = w * 128 + slot_in_w[w]
                slot_in_w[w] += 1
            node_local[lo:hi] = newloc

    # ---- chunk decomposition: 4 window-chunks, sized so per-(window, chunk)
    # gather runs land just under multiples of 128, and each chunk's block of
    # 8*128*w_q table rows stays within int16 index range. ----
    NBLK = cfg["NBLK"]
    mean_w = max(1.0, (deg_td.sum() + deg_bu.sum()) / (2.0 * C * W)) if deg_td is not None else 128.0
    wmax = min(W, (32767 // (128 * C)))

    def padfrac(wb):
        r = wb / W * mean_w  # mean edges per (window, this-chunk) run
        if r <= 0:
            return 0.0
        margin = 1.6 * np.sqrt(r) + 6
        gslots = 128 * np.ceil((r + margin) / 128)
        return (gslots - r) * 1.0

    best = None
    for w1 in range(1, wmax + 1):
        for w2 in range(w1, wmax + 1):
            for w3 in range(w2, wmax + 1):
                w4 = W - w1 - w2 - w3
                if w4 < w3 or w4 > wmax:
                    continue
                cost = padfrac(w1) + padfrac(w2) + padfrac(w3) + padfrac(w4)
                if best is None or cost < best[0]:
                    best = (cost, (w1, w2, w3, w4))
    ws = sorted(best[1]) if best else [W]
    # small chunks first: their table writes complete earliest, letting
    # the gather phase start sooner
    cw = np.concatenate([[0], np.cumsum(ws)])
    assert cw[-1] == W

    chunk_of_w = np.searchsorted(cw[1:], np.arange(W), side="right")
    q = chunk_of_w[np.minimum(node_local // 128, W - 1)]
    rpr = 128 * np.diff(cw)  # rows per rank per chunk
    base = np.concatenate([[0], np.cumsum(rpr * C)])
    table_row = base[q] + node_core * rpr[q] + (node_local - 128 * cw[q])
    bounds = [int(b) for b in base]
    return dict(starts=starts, counts=counts, NPC=NPC, gpc=gpc,
                node_core=node_core.astype(np.int64),
                node_local=node_local.astype(np.int64),
                table_row=table_row.astype(np.int64),
                cw=cw, bounds=bounds)


def build_direction_meta(gather_nodes, target_nodes, part, cfg):
    """Build per-core gather index / dstloc arrays and the uniform group
    structure for one edge direction.  Self-loops v->v are appended to the
    edge list so the epilogue needs no separate self term.  Slots within each
    (super, block, window) run are sorted by target position so each group's
    targets span a narrow band; per group a band (b0, wid in {64,128}) is
    chosen uniformly across cores and dstloc is stored band-relative.
    """
    N, C = cfg["N"], cfg["N_CORES"]
    SW, NBLK = cfg["SW"], cfg["NBLK"]
    NPC = part["NPC"]
    W = NPC // 128
    NS = (W + SW - 1) // SW

    loop = np.arange(N, dtype=np.int64)
    gather_nodes = np.concatenate([np.asarray(gather_nodes), loop])
    target_nodes = np.concatenate([np.asarray(target_nodes), loop])

    deg = np.bincount(target_nodes, minlength=N).astype(np.float64)

    bounds = part["bounds"]
    assert len(bounds) == NBLK + 1
    assert all(bounds[i + 1] - bounds[i] <= 32767 for i in range(NBLK))
    bounds_arr = np.array(bounds[1:-1])

    tr_g = part["table_row"][gather_nodes]
    t_core = part["node_core"][target_nodes]
    t_local = part["node_local"][target_nodes]
    lw = t_local // 128          # window
    dloc = t_local % 128         # position within window
    blk = np.searchsorted(bounds_arr, tr_g, side="right")
    idxv = tr_g - np.array(bounds[:-1])[blk]
    sup = lw // SW

    # per (core, s, b, w) counts -> uniform G
    keyW = (sup * NBLK + blk) * W + lw  # key within a core
    nkeys = NS * NBLK * W
    counts = np.zeros((C, nkeys), np.int64)
    for c in range(C):
        m = t_core == c
        counts[c] = np.bincount(keyW[m], minlength=nkeys)
    max_counts = counts.max(axis=0).reshape(NS, NBLK, W)

    G = np.ceil(max_counts / 128).astype(np.int64)  # groups per (s,b,w)
    # ensure every window has at least one group (psum must be written)
    for s in range(NS):
        w_lo, w_hi = s * SW, min((s + 1) * SW, W)
        for w in range(w_lo, w_hi):
            if G[s, :, w].sum() == 0:
                G[s, 0, w] = 1
        G[s, :, :w_lo] = 0
        G[s, :, w_hi:] = 0
    Gflat = G.reshape(nkeys)
    gmax = int(Gflat.max())

    # ---- per-core sorted runs (sorted by dloc within each key run) ----
    percore = []
    lo_all = np.full((nkeys, gmax), 128, np.int64)
    hi_all = np.full((nkeys, gmax), -1, np.int64)
    for c in range(C):
        m = t_core == c
        k = keyW[m]
        order = np.lexsort((dloc[m], k))
        ks = k[order]
        dvs = dloc[m][order]
        ivs = idxv[m][order]
        run_start = np.searchsorted(ks, np.arange(nkeys))
        rank = np.arange(len(ks)) - run_start[ks]
        percore.append((ks, dvs, ivs, rank))
        grp = np.minimum(rank // 128, gmax - 1)
        np.minimum.at(lo_all, (ks, grp), dvs)
        np.maximum.at(hi_all, (ks, grp), dvs)

    # 64-wide band, base in {0,64} (PSUM partition-base constraint: base 0/64
    # for 64-wide writes; base 32 allows only 32-wide)
    b0c = np.where(lo_all >= 64, 64, 0)
    fits = (hi_all < 0) | (hi_all <= b0c + 63)
    wid_all = np.where(fits, 64, 128)
    b0_all = np.where(fits & (hi_all >= 0), b0c, 0)

    # structure: per (s,b): window col bases, totals, group band records
    struct = []
    for s in range(NS):
        w_lo, w_hi = s * SW, min((s + 1) * SW, W)
        for b in range(NBLK):
            g_list = G[s, b, w_lo:w_hi]
            base = np.concatenate([[0], np.cumsum(g_list)])
            struct.append(dict(s=s, b=b, w_lo=w_lo, w_hi=w_hi,
                               g_list=g_list, g_base=base,
                               G=int(g_list.sum())))
    # global group/column offsets + per-group band records
    offG = 0
    off16 = 0
    n64 = n128 = 0
    for sb in struct:
        sb["offG"] = offG
        sb["off16"] = off16
        groups = []
        s, b = sb["s"], sb["b"]
        for i, w in enumerate(range(sb["w_lo"], sb["w_hi"])):
            key = (sb["s"] * NBLK + b) * W + w
            for g in range(int(sb["g_list"][i])):
                wid = int(wid_all[key, g])
                b0 = int(b0_all[key, g])
                if wid == 64:
                    groups.append((w, 64, n64, b0))
                    n64 += 1
                else:
                    groups.append((w, 128, n128, b0))
                    n128 += 1
        sb["groups"] = groups
        offG += sb["G"]
        off16 += sb["G"] * 8  # 128 slots / 16
    CG = offG
    CG64, CG128 = n64, n128
    Gmax = max((sb["G"] for sb in struct), default=1)

    # per-super class offsets (for dl tile loads)
    supers = []
    for si in range(NS):
        sbs = struct[si * NBLK:(si + 1) * NBLK]
        g64 = [g for sb in sbs for g in sb["groups"] if g[1] == 64]
        g128 = [g for sb in sbs for g in sb["groups"] if g[1] == 128]
        supers.append(dict(
            off64=g64[0][2] if g64 else 0, cnt64=len(g64),
            off128=g128[0][2] if g128 else 0, cnt128=len(g128)))

    # ---- per-edge slot assignment (per core) ----
    idx_all = np.zeros((C, 128, CG * 8), np.int16)
    dl64_all = np.full((C, 128, max(CG64, 1)), -1.0, BF16)
    dl128_all = np.full((C, 128, max(CG128, 1)), -1.0, BF16)
    # per (key, g): global slot base, class, class idx, b0
    slot_base = np.zeros((nkeys, gmax), np.int64)
    cls_arr = np.zeros((nkeys, gmax), np.int64)
    cidx_arr = np.zeros((nkeys, gmax), np.int64)
    for sb in struct:
        s, b = sb["s"], sb["b"]
        gi = 0
        for i, w in enumerate(range(sb["w_lo"], sb["w_hi"])):
            key = (s * NBLK + b) * W + w
            for g in range(int(sb["g_list"][i])):
                gslot = (sb["offG"] + sb["g_base"][i] + g) * 128
                slot_base[key, g] = gslot
                _, cls, cidx, _ = sb["groups"][gi]
                cls_arr[key, g] = cls
                cidx_arr[key, g] = cidx
                gi += 1

    for c in range(C):
        ks, dvs, ivs, rank = percore[c]
        grp = np.minimum(rank // 128, gmax - 1)
        slot = slot_base[ks, grp] + (rank - grp * 128)
        dshift = dvs - b0_all[ks, grp]
        # idx wrapped layout: slot j -> (j%16, j//16), replicated x8
        prow = slot % 16
        pcol = slot // 16
        tmp = np.zeros((16, CG * 8), np.int16)
        tmp[prow, pcol] = ivs.astype(np.int16)
        idx_all[c] = np.tile(tmp, (8, 1))
        cls_s = cls_arr[ks, grp]
        cidx_s = cidx_arr[ks, grp]
        srow = slot % 128
        m64 = cls_s == 64
        dl64_all[c, srow[m64], cidx_s[m64]] = dshift[m64].astype(BF16)
        m128 = ~m64
        dl128_all[c, srow[m128], cidx_s[m128]] = dshift[m128].astype(BF16)

    return dict(deg=deg, struct=struct, supers=supers, CG=CG,
                CG64=max(CG64, 1), CG128=max(CG128, 1), Gmax=Gmax, NS=NS, W=W,
                bounds=bounds, idx_all=idx_all,
                dl64_all=dl64_all, dl128_all=dl128_all)


def build_Mp(src, dst, batch, part, td_deg, bu_deg, cfg):
    """Folded layer-2 coefficients, local-row form, fp8 with MP_SCALE folded.

    out_graph[g] = (sum_s Mp[s,g] * h1[s]) @ W2 + n_g * b2, with the sum over
    LOCAL nodes s of each core (rows in node_local order)."""
    C, N, G = cfg["N_CORES"], cfg["N"], cfg["NUM_GRAPHS"]
    NPC = part["NPC"]
    nc_ = part["node_core"]
    nl = part["node_local"]
    batch = np.asarray(batch)

    dinv_td = (1.0 / np.sqrt(td_deg)).astype(np.float32)
    dinv_bu = (1.0 / np.sqrt(bu_deg)).astype(np.float32)

    M_td = np.zeros((C, NPC, G), np.float32)
    M_bu = np.zeros((C, NPC, G), np.float32)
    # td: value row src, target dst -> coeff dinv_td[dst] at (core(src), loc(src), g(dst))
    np.add.at(M_td, (nc_[src], nl[src], batch[dst]), dinv_td[dst])
    # bu: value row dst, target src -> coeff dinv_bu[src]
    np.add.at(M_bu, (nc_[dst], nl[dst], batch[src]), dinv_bu[src])
    # self-loop diagonals
    allv = np.arange(N)
    np.add.at(M_td, (nc_[allv], nl[allv], batch[allv]), dinv_td[allv])
    np.add.at(M_bu, (nc_[allv], nl[allv], batch[allv]), dinv_bu[allv])
    # fold the value-side dinv (from hn2 = dinv * (h1@W2)) into M
    f_td = np.zeros((C, NPC), np.float32)
    f_bu = np.zeros((C, NPC), np.float32)
    f_td[nc_[allv], nl[allv]] = dinv_td[allv]
    f_bu[nc_[allv], nl[allv]] = dinv_bu[allv]
    M_td *= f_td[:, :, None] * cfg["MP_SCALE"]
    M_bu *= f_bu[:, :, None] * cfg["MP_SCALE"]
    n_g = np.bincount(batch, minlength=G).astype(np.float32)
    return dict(Mp_td=M_td.astype(FP8), Mp_bu=M_bu.astype(FP8), n_g=n_g,
                dinv_td=dinv_td, dinv_bu=dinv_bu)


def build_all_inputs(x, edge_index, batch, Ws, bs, cfg):
    """Produce per-core in_maps plus structural metadata."""
    C = cfg["N_CORES"]
    N = cfg["N"]
    src = np.asarray(edge_index[0])
    dst = np.asarray(edge_index[1])
    part = build_partition(batch, cfg,
                           deg_td=np.bincount(dst, minlength=N),
                           deg_bu=np.bincount(src, minlength=N))
    NPC = part["NPC"]
    W = NPC // 128
    R = C * NPC
    RW = R // 128

    td = build_direction_meta(src, dst, part, cfg)   # gather src row, scatter to dst
    bu = build_direction_meta(dst, src, part, cfg)   # reversed
    mp = build_Mp(src, dst, batch, part, td["deg"], bu["deg"], cfg)

    PIECE_G = cfg["PIECE_G"]
    iota128 = np.tile(np.arange(128, dtype=np.float32), PIECE_G)[None, :] \
        .repeat(128, 0).astype(BF16)
    iota64 = np.tile(np.arange(64, dtype=np.float32), PIECE_G)[None, :] \
        .repeat(128, 0).astype(BF16)

    # global fp8 x in table-row order + global per-row dinv (both directions
    # interleaved) used to scale the A1 psum before the fp8 table store
    tr = part["table_row"]
    xTg = np.zeros((256, R), FP8)
    xTg[:, tr] = np.asarray(x).T.astype(FP8)
    dinvg = np.ones((R, 2), np.float32)
    dinvg[tr, 0] = mp["dinv_td"]
    dinvg[tr, 1] = mp["dinv_bu"]
    dinvg = np.ascontiguousarray(
        dinvg.reshape(RW, 128, 2).transpose(1, 0, 2).reshape(128, RW * 2))
    W1cat = np.concatenate([Ws[0], Ws[2]], axis=1).astype(FP8)  # [256, 256]

    ngb2 = np.concatenate([np.outer(mp["n_g"], bs[1]),
                           np.outer(mp["n_g"], bs[3])], axis=1).astype(np.float32)

    # per-core tensors
    in_maps = []
    for c in range(C):
        lo, hi = part["starts"][c], part["starts"][c + 1]
        li = part["node_local"][lo:hi]
        dloc = np.ones((128, W, 2), np.float32)
        dloc[li % 128, li // 128, 0] = mp["dinv_td"][lo:hi]
        dloc[li % 128, li // 128, 1] = mp["dinv_bu"][lo:hi]
        im = dict(
            xTg=xTg,
            dinvg=dinvg,
            W1cat=W1cat,
            dinv_loc=dloc.reshape(128, W * 2),
            iota64=iota64, iota128=iota128,
            Mp_td=mp["Mp_td"][c], Mp_bu=mp["Mp_bu"][c],
            idx_td=td["idx_all"][c], idx_bu=bu["idx_all"][c],
            dl64_td=td["dl64_all"][c], dl128_td=td["dl128_all"][c],
            dl64_bu=bu["dl64_all"][c], dl128_bu=bu["dl128_all"][c],
            W_td2=(Ws[1] / cfg["MP_SCALE"]).astype(BF16),
            W_bu2=(Ws[3] / cfg["MP_SCALE"]).astype(BF16),
            b_td1=np.tile(bs[0][None, :], (128, 1)).astype(np.float32),
            b_bu1=np.tile(bs[2][None, :], (128, 1)).astype(np.float32),
        )
        in_maps.append(im)
    meta = dict(part=part, td=td, bu=bu, NPC=NPC, W=W, cfg=cfg,
                R=R, RW=RW, ngb2=ngb2)
    return in_maps, meta


# =====================================================================
# Bass program
# =====================================================================

def build_bass(meta):
    import concourse.bacc as bacc
    import concourse.mybir as mybir
    import concourse.tile as tile

    cfg = meta["cfg"]
    C = cfg["N_CORES"]
    NPC, W = meta["NPC"], meta["W"]
    IN, HID = cfg["IN_FEATS"], cfg["HIDDEN"]
    NBLK = cfg["NBLK"]
    NG = cfg["NUM_GRAPHS"]
    PIECE_G = cfg["PIECE_G"]
    R, RW = meta["R"], meta["RW"]
    f32, bf16, i16 = mybir.dt.float32, mybir.dt.bfloat16, mybir.dt.int16
    fp8 = mybir.dt.float8e4
    DR = mybir.MatmulPerfMode.DoubleRow

    nc = bacc.Bacc("TRN2", target_bir_lowering=False, debug=False, num_devices=C,
                   num_swdge_queues=4)

    # ---- I/O ----
    ten = {}
    def inp(name, shape, dt):
        ten[name] = nc.dram_tensor(name, shape, dt, kind="ExternalInput")
        return ten[name]

    inp("dinv_loc", [128, W * 2], f32)
    inp("iota64", [128, PIECE_G * 64], bf16)
    inp("iota128", [128, PIECE_G * 128], bf16)
    inp("xTg", [IN, R], fp8)
    inp("dinvg", [128, RW * 2], f32)
    inp("W1cat", [IN, 2 * HID], fp8)
    for d in ("td", "bu"):
        m = meta[d]
        inp(f"idx_{d}", [128, m["CG"] * 8], i16)
        inp(f"dl64_{d}", [128, m["CG64"]], bf16)
        inp(f"dl128_{d}", [128, m["CG128"]], bf16)
        inp(f"Mp_{d}", [NPC, NG], fp8)
        inp(f"W_{d}2", [HID, HID], bf16)
        inp(f"b_{d}1", [128, HID], f32)
    out_t = nc.dram_tensor("out", [NG, 2 * HID], f32, kind="ExternalOutput")

    # internal DRAM: per-block packed hn tables [rows_b, 256] fp8
    bounds = meta["td"]["bounds"]
    table = {}
    for b in range(NBLK):
        table[b] = nc.dram_tensor(
            f"table{b}", [bounds[b + 1] - bounds[b], 2 * HID], fp8,
            kind="Internal")

    from contextlib import ExitStack
    with tile.TileContext(nc) as tc, ExitStack() as stack:
        def pool(name, bufs, space="SBUF"):
            return stack.enter_context(tc.tile_pool(name=name, bufs=bufs, space=space))

        const = pool("const", 1)
        xt_p = pool("xt", 6)
        hnb_p = pool("hnb", 3)
        idx_p = pool("idx", 6)
        dl_p = pool("dl", 6)
        gat_p = pool("gat", 10)              # gathered edge tiles (fp8)
        oh_p = pool("oh", 5)                 # one-hot tiles (fp8)
        mp_p = pool("mp", 3)                 # Mp super tiles
        win_p = pool("win", 4, "PSUM")       # window psum, 4 windows/bank
        pps_p = pool("pps", 3, "PSUM")       # A1 psum pairs / P-partial halves
        hps_p = pool("hps", 1, "PSUM")       # final projection psum
        epi_p = pool("epi", 6)               # epilogue sbuf tiles
        h1_p = pool("h1", 4)
        accs = pool("accs", 1)               # P accumulator (SBUF, f32)

        # ---- constants in SBUF ----
        iota_t = {}
        for wd, cols in ((64, PIECE_G * 64), (128, PIECE_G * 128)):
            t = const.tile([128, cols], bf16, tag=f"iota{wd}")
            nc.sync.dma_start(t[:], ten[f"iota{wd}"][:])
            iota_t[wd] = t
        W1c = []
        for kk in range(IN // 128):
            t = const.tile([128, 2 * HID], fp8, tag=f"W1c{kk}", name=f"W1c{kk}")
            nc.sync.dma_start(t[:], ten["W1cat"][kk * 128:(kk + 1) * 128, :])
            W1c.append(t)
        dinvg_t = const.tile([128, RW * 2], f32, tag="dinvg")
        nc.sync.dma_start(dinvg_t[:], ten["dinvg"][:])
        W2t, bt = {}, {}
        for d in ("td", "bu"):
            t = const.tile([128, HID], bf16, tag=f"W2_{d}", name=f"W2_{d}")
            nc.sync.dma_start(t[:], ten[f"W_{d}2"][:])
            W2t[d] = t
            t = const.tile([128, HID], f32, tag=f"b_{d}1", name=f"bt_{d}1")
            nc.sync.dma_start(t[:], ten[f"b_{d}1"][:])
            bt[d] = t
        dinvl_t = const.tile([128, W * 2], f32, tag="dinvl")
        nc.sync.dma_start(dinvl_t[:], ten["dinv_loc"][:])
        zrow = const.tile([1, 512], bf16, tag="zrow")
        nc.gpsimd.memset(zrow[:], 0.0)

        # P accumulator [128f, td 1024g | bu 1024g] f32
        acc = accs.tile([128, 2 * NG], f32, tag="acc", name="acc")
        nc.gpsimd.memset(acc[:], 0.0)

        cw = meta["part"]["cw"]
        chunk_base_oct = [int(cw[q]) for q in range(NBLK + 1)]  # x8 windows = octets

        # ---- A1 (replicated): hn = dinv * (x @ W1), both directions packed,
        # written chunk-major to the fp8 tables ----
        nK = IN // 128

        def emit_a1_octet(oct_i):
            q = 0
            while chunk_base_oct[q + 1] * 8 <= oct_i * 8:
                q += 1
            xts = []
            for kk in range(nK):
                t = xt_p.tile([128, 8 * 128], fp8, tag="xt", name=f"xa_{oct_i}_{kk}")
                nc.sync.dma_start(
                    t[:], ten["xTg"][kk * 128:(kk + 1) * 128,
                                     oct_i * 1024:(oct_i + 1) * 1024])
                xts.append(t)
            hnb = hnb_p.tile([128, 8, 2 * HID], fp8, tag="hnb", name=f"hnb_{oct_i}")
            for j0 in range(0, 8, 2):
                hps = pps_p.tile([128, 2, 2 * HID], f32, tag="pps")
                for j in (j0, j0 + 1):
                    for kk in range(nK):
                        nc.tensor.matmul(hps[:, j - j0, :],
                                         xts[kk][:, j * 128:(j + 1) * 128],
                                         W1c[kk][:], start=(kk == 0),
                                         stop=(kk == nK - 1),
                                         skip_group_check=True)
                c0 = (oct_i * 8 + j0) * 2
                nc.vector.tensor_tensor(
                    out=hnb[:, j0:j0 + 2, :].rearrange("p j (t f) -> p (j t) f", f=HID),
                    in0=hps[:].rearrange("p j (t f) -> p (j t) f", f=HID),
                    in1=dinvg_t[:, c0:c0 + 4]
                    .rearrange("p (x o) -> p x o", o=1)
                    .to_broadcast([128, 4, HID]),
                    op=mybir.AluOpType.mult)
            rowb = oct_i * 1024 - bounds[q]
            nc.scalar.dma_start(
                table[q][rowb:rowb + 1024, :].rearrange("(j p) f -> p j f", p=128),
                hnb[:])

        # ---- edge-phase per-super loads (idx/dl on sync pre-A1 for the first
        # supers, then on the gpsimd queue so they never sit behind A1 DMA) ----
        sup_tiles = {}

        def emit_super_loads(d, si, eng):
            m = meta[d]
            structs = m["struct"]
            sb0 = structs[si * NBLK]
            supG = sum(x["G"] for x in structs[si * NBLK:(si + 1) * NBLK])
            rec = m["supers"][si]
            ts = {}
            if supG > 0:
                t = idx_p.tile([128, supG * 8], i16, tag="idx")
                eng.dma_start(t[:], ten[f"idx_{d}"]
                              [:, sb0["off16"]:sb0["off16"] + supG * 8])
                ts["idx"] = t
            for wd in (64, 128):
                cnt = rec[f"cnt{wd}"]
                if cnt > 0:
                    t = dl_p.tile([128, cnt], bf16, tag=f"dl{wd}")
                    off = rec[f"off{wd}"]
                    eng.dma_start(t[:], ten[f"dl{wd}_{d}"][:, off:off + cnt])
                    ts[f"dl{wd}"] = t
            sup_tiles[(d, si)] = ts

        PREF = 3
        for si in range(min(PREF, meta["td"]["NS"])):
            for d in ("td", "bu"):
                emit_super_loads(d, si, nc.sync)

        for oct_i in range(RW // 8):
            emit_a1_octet(oct_i)

        # ---- edge phase (layer-1 aggregation + fused pooled layer-2) ----
        qn = [0]

        def epilogue(d, w, pt, mpt, wi, pps, w_lo, w_hi):
            di = 0 if d == "td" else 1
            o1 = epi_p.tile([128, HID], f32, tag="o1")
            nc.vector.scalar_tensor_tensor(
                out=o1[:], in0=pt[:], scalar=dinvl_t[:, w * 2 + di:w * 2 + di + 1],
                in1=bt[d][:],
                op0=mybir.AluOpType.mult, op1=mybir.AluOpType.add)
            h1 = h1_p.tile([128, HID], fp8, tag="h1")
            nc.scalar.activation(h1[:], o1[:], mybir.ActivationFunctionType.Relu)
            for h in range(2):
                nc.tensor.matmul(pps[h][:], h1[:], mpt[:, wi, h * 512:(h + 1) * 512],
                                 start=(w == w_lo), stop=(w == w_hi - 1),
                                 skip_group_check=True)

        def edge_phase(d):
            m = meta[d]
            doff = 0 if d == "td" else HID
            structs = m["struct"]
            supers = m["supers"]
            last_mm = {}
            for sbi, sb in enumerate(structs):
                gi = 0
                for i, w in enumerate(range(sb["w_lo"], sb["w_hi"])):
                    for g in range(int(sb["g_list"][i])):
                        if sb["g_list"][i] > 0:
                            last_mm[w] = (sbi, int(sb["g_base"][i]) + g)
            quad_tiles = {}
            def win_q(w):
                q = w // 4
                if q not in quad_tiles:
                    qt = win_p.tile([128, 512], f32, tag="win",
                                    name=f"win_{d}_{q}")
                    nc.tensor.matmul(qt[:], zrow[0:1, 0:128], zrow[0:1, 0:512],
                                     start=True, stop=False, skip_group_check=True)
                    quad_tiles[q] = qt
                return quad_tiles[q]
            it_sup = None
            dlt = {}
            sup_rec = None
            sup_off16 = 0
            for sbi, sb in enumerate(structs):
                si = sbi // NBLK
                if sbi % NBLK == 0:
                    sup_off16 = sb["off16"]
                    sup_rec = supers[si]
                    if (d, si) not in sup_tiles:
                        emit_super_loads(d, si, nc.scalar)
                    ts = sup_tiles[(d, si)]
                    it_sup = ts.get("idx")
                    dlt = {64: ts.get("dl64"), 128: ts.get("dl128")}
                    # Mp tile for the whole super (both layer-2 halves)
                    nsw = sb["w_hi"] - sb["w_lo"]
                    mpt = mp_p.tile([128, nsw, NG], fp8, tag="mp",
                                    name=f"mp_{d}_{si}")
                    nc.scalar.dma_start(
                        mpt[:], ten[f"Mp_{d}"][sb["w_lo"] * 128:sb["w_hi"] * 128, :]
                        .rearrange("(j p) g -> p j g", p=128))
                    sup_mpt = mpt
                G = sb["G"]
                if G == 0:
                    continue
                r16 = sb["off16"] - sup_off16
                # flat (window, group-record) list for this (s,b)
                wg = []
                gi = 0
                for i, w in enumerate(range(sb["w_lo"], sb["w_hi"])):
                    for g in range(int(sb["g_list"][i])):
                        wg.append((w, int(sb["g_base"][i]) + g) + sb["groups"][gi])
                        gi += 1
                # split into pieces of <= PIECE_G groups
                npiece = (G + PIECE_G - 1) // PIECE_G
                for pi in range(npiece):
                    g0 = pi * G // npiece
                    g1 = (pi + 1) * G // npiece
                    pg = g1 - g0
                    gt = gat_p.tile([128, pg, 2 * HID], fp8, tag="gat")
                    qn[0] += 1
                    nc.gpsimd.dma_gather(gt[:], table[sb["b"]][:],
                                         it_sup[:, r16 + g0 * 8:r16 + g1 * 8],
                                         num_idxs=pg * 128,
                                         num_idxs_reg=pg * 128, elem_size=2 * HID,
                                         single_packet=False, queue_num=qn[0] % 4)
                    # one-hots per band class
                    ohs = {}
                    cls_lo = {}
                    for wd in (64, 128):
                        recs = [r for r in wg[g0:g1] if r[3] == wd]
                        if not recs:
                            continue
                        c_lo = recs[0][4]
                        ncl = len(recs)
                        cls_lo[wd] = c_lo
                        oh = oh_p.tile([128, PIECE_G * wd], fp8, tag=f"oh{wd}")
                        ohs[wd] = oh
                        s_off = sup_rec[f"off{wd}"]
                        nc.vector.tensor_tensor(
                            out=oh[:, :ncl * wd].rearrange("p (g f) -> p g f", f=wd),
                            in0=dlt[wd][:, c_lo - s_off:c_lo - s_off + ncl]
                            .rearrange("p (g o) -> p g o", o=1)
                            .to_broadcast([128, ncl, wd]),
                            in1=iota_t[wd][:, :ncl * wd]
                            .rearrange("p (g f) -> p g f", f=wd),
                            op=mybir.AluOpType.is_equal)
                    for (w, g, _w2, wd, cidx, b0) in wg[g0:g1]:
                        qt = win_q(w)
                        ci = cidx - cls_lo[wd]
                        nc.tensor.matmul(
                            qt[b0:b0 + wd, (w % 4) * 128:(w % 4 + 1) * 128],
                            ohs[wd][:, ci * wd:(ci + 1) * wd],
                            gt[:, g - g0, doff:doff + HID],
                            start=False, stop=(last_mm[w] == (sbi, g)),
                            skip_group_check=True)
                # epilogues for completed supers: after last block of super
                if sb["b"] == NBLK - 1:
                    pps = [pps_p.tile([128, 512], f32, tag="pps",
                                      name=f"pps{h}_{d}_{sb['s']}")
                           for h in range(2)]
                    for wi, w in enumerate(range(sb["w_lo"], sb["w_hi"])):
                        pt = win_q(w)[:, (w % 4) * 128:(w % 4 + 1) * 128]
                        epilogue(d, w, pt, sup_mpt, wi, pps, sb["w_lo"], sb["w_hi"])
                    # fold the super's P-partial into the SBUF accumulator
                    aoff = 0 if d == "td" else NG
                    for h in range(2):
                        nc.vector.tensor_tensor(
                            out=acc[:, aoff + h * 512:aoff + (h + 1) * 512],
                            in0=acc[:, aoff + h * 512:aoff + (h + 1) * 512],
                            in1=pps[h][:], op=mybir.AluOpType.add)
                    quad_tiles.clear()
                    yield sb["w_hi"]
                else:
                    yield None

        def run_layer():
            gens = {"td": edge_phase("td"), "bu": edge_phase("bu")}
            done = {"td": False, "bu": False}
            while not all(done.values()):
                for d in ("td", "bu"):
                    if done[d]:
                        continue
                    try:
                        next(gens[d])
                    except StopIteration:
                        done[d] = True

        run_layer()

        # ---- final projection: out_partial[g] = P^T @ W2 (host sums partials) ----
        for d, (aoff, ooff) in (("td", (0, 0)), ("bu", (NG, HID))):
            for gc in range(NG // 128):
                pb = epi_p.tile([128, 128], bf16, tag="pb")
                nc.vector.tensor_copy(pb[:], acc[:, aoff + gc * 128:aoff + (gc + 1) * 128])
                fps = hps_p.tile([128, 4, HID], f32, tag="hps")
                nc.tensor.matmul(fps[:, 0, :], pb[:], W2t[d][:], start=True, stop=True,
                                 skip_group_check=True)
                ob = epi_p.tile([128, HID], f32, tag="ob")
                nc.vector.tensor_copy(ob[:], fps[:, 0, :])
                nc.sync.dma_start(out_t[gc * 128:(gc + 1) * 128, ooff:ooff + HID], ob[:])

    nc.compile()
    return nc


# =====================================================================
# Entry point
# =====================================================================

def _run(inputs, cfg, trace=False):
    from concourse import bass_utils
    x = np.asarray(inputs["x"], np.float32)
    edge_index = np.asarray(inputs["edge_index"])
    batch = np.asarray(inputs["batch"])
    Ws = [np.asarray(inputs[k], np.float32) for k in ("W_td1", "W_td2", "W_bu1", "W_bu2")]
    bs = [np.asarray(inputs[k], np.float32) for k in ("b_td1", "b_td2", "b_bu1", "b_bu2")]
    in_maps, meta = build_all_inputs(x, edge_index, batch, Ws, bs, cfg)
    nc = build_bass(meta)
    res = bass_utils.run_bass_kernel_spmd(
        nc, in_maps, core_ids=list(range(cfg["N_CORES"])), trace=trace)
    out = sum(res.results[c]["out"].astype(np.float64) for c in range(cfg["N_CORES"]))
    out = out + meta["ngb2"].astype(np.float64)
    return out.astype(np.float32), res


def kernel(**inputs):
    out, _ = _run(inputs, FULL_CFG, trace=False)
    return out


# revision 21
# speedup vs baseline: 1.4518x; 1.1635x over previous
"""BiGCN (2-layer bidirectional GCN + global add pool) on 8 Trainium2 NeuronCores.

Strategy (hardcoded for the nn_BiGCN_graphcl problem shapes):
  - Nodes are sharded graph-aligned: core c owns graphs [128c, 128c+128) and
    their (contiguous, batch-sorted) node range, padded to a common NPC.
  - Layer-1 node features hn1 = dinv * (x @ W1) are computed REPLICATED: every
    core computes the full [R, 256] table locally from globally reordered,
    dinv-prescaled fp8 copies of x (one per direction), using fp8 DoubleRow
    matmuls (K=256 in one instruction).  NO collectives at all.  Table rows
    pack both directions: row = [hn_td | hn_bu] in fp8 (256 bytes), written
    chunk-major so the edge phase can start as soon as chunk 0 lands.
  - Self-loops are folded into the edge lists (an extra edge v->v per node),
    so the epilogue is just out = dinv * agg + b with no separate hn term.
  - Per direction (td / bu), edges are assigned to the core owning their
    target node.  Each core gathers 256B packed rows for its edge shard with
    dma_gather (4 SWDGE queues) and segment-sums on the TensorEngine into
    per-window (128-node) PSUM tiles (fp8 x fp8 matmuls).  Slots are sorted
    by target position within each (window, block) run, so each 128-slot
    group's targets span a narrow band: the one-hot (DVE is_equal against an
    iota constant, fp8 out) is built only band-wide (64 cols for ~98% of
    groups, 128 otherwise), halving DVE work.
  - The final output is graph-pooled, so layer 2 collapses algebraically:
        out[g] = (sum_s Mp[s,g] * h1[s]) @ W2 + n_g * b2
    with Mp host-precomputed in fp8 (x8 scaled; W2 carries the /8).  Each
    core contracts its LOCAL h1 rows against Mp into a [128f, 1024g] partial
    accumulator; the host sums the 8 per-core partial outputs.
  - The SPMD program is identical on all cores: all per-core variation lives
    in uploaded index/data tensors; run lengths are padded to the max across
    cores (pad slots gather row 0 of the block and carry dstloc=-1 so their
    one-hot column is zero).
"""

import math
import numpy as np
import ml_dtypes

BF16 = ml_dtypes.bfloat16
FP8 = ml_dtypes.float8_e4m3

# ---------------------------------------------------------------- problem cfg
FULL_CFG = dict(
    N=100000, E=1600000, IN_FEATS=256, HIDDEN=128, OUT_FEATS=128,
    NUM_GRAPHS=1024, N_CORES=8, SW=8, NBLK=4, PIECE_G=32, MP_SCALE=8.0,
)


def _round_up(x, m):
    return (x + m - 1) // m * m


# =====================================================================
# Host-side metadata construction
# =====================================================================

def build_partition(batch, cfg, deg_td=None, deg_bu=None):
    """Graph-aligned node partition. Returns dict with per-core node ranges.

    If degree arrays are given, each core's local node order is permuted so
    that per-window (128-node) degree sums cluster just under multiples of
    4*128 edges per (window, src-block) run, minimizing ceil-128 padding."""
    N, C, G = cfg["N"], cfg["N_CORES"], cfg["NUM_GRAPHS"]
    gpc = G // C  # graphs per core
    starts = np.searchsorted(batch, np.arange(0, G + 1, gpc))
    counts = np.diff(starts)
    NPC = max(128, _round_up(int(counts.max()), 128))
    W = NPC // 128
    node_core = np.searchsorted(starts[1:], np.arange(N), side="right")
    node_local = np.arange(N) - starts[node_core]

    if deg_td is not None:
        for c in range(C):
            lo, hi = starts[c], starts[c + 1]
            cnt = hi - lo
            dt = deg_td[lo:hi].astype(np.int64)
            db = deg_bu[lo:hi].astype(np.int64)
            order = np.argsort(-(dt + db), kind="stable")
            tg_t = np.full(W, dt.sum() / W)
            tg_b = np.full(W, db.sum() / W)
            rem_t = tg_t.astype(np.float64).copy()
            rem_b = tg_b.astype(np.float64).copy()
            room = np.full(W, 128, np.int64)
            assign = np.empty(cnt, np.int64)
            for j in order:
                score = np.minimum(rem_t - dt[j], rem_b - db[j])
                score[room <= 0] = -np.inf
                w = int(np.argmax(score))
                assign[j] = w
                rem_t[w] -= dt[j]
                rem_b[w] -= db[j]
                room[w] -= 1
            # positions: window-major order
            slot_in_w = np.zeros(W, np.int64)
            newloc = np.empty(cnt, np.int64)
            for j in range(cnt):
                w = assign[j]
                newloc[j] = w * 128 + slot_in_w[w]
                slot_in_w[w] += 1
            node_local[lo:hi] = newloc

    # ---- chunk decomposition: 4 window-chunks, sized so per-(window, chunk)
    # gather runs land just under multiples of 128, and each chunk's block of
    # 8*128*w_q table rows stays within int16 index range. ----
    NBLK = cfg["NBLK"]
    mean_w = max(1.0, (deg_td.sum() + deg_bu.sum()) / (2.0 * C * W)) if deg_td is not None else 128.0
    wmax = min(W, (32767 // (128 * C)))

    def padfrac(wb):
        r = wb / W * mean_w  # mean edges per (window, this-chunk) run
        if r <= 0:
            return 0.0
        margin = 1.6 * np.sqrt(r) + 6
        gslots = 128 * np.ceil((r + margin) / 128)
        return (gslots - r) * 1.0

    best = None
    for w1 in range(1, wmax + 1):
        for w2 in range(w1, wmax + 1):
            for w3 in range(w2, wmax + 1):
                w4 = W - w1 - w2 - w3
                if w4 < w3 or w4 > wmax:
                    continue
                cost = padfrac(w1) + padfrac(w2) + padfrac(w3) + padfrac(w4)
                if best is None or cost < best[0]:
                    best = (cost, (w1, w2, w3, w4))
    ws = sorted(best[1]) if best else [W]
    # small chunks first: their table writes complete earliest, letting
    # the gather phase start sooner
    cw = np.concatenate([[0], np.cumsum(ws)])
    assert cw[-1] == W

    chunk_of_w = np.searchsorted(cw[1:], np.arange(W), side="right")
    q = chunk_of_w[np.minimum(node_local // 128, W - 1)]
    rpr = 128 * np.diff(cw)  # rows per rank per chunk
    base = np.concatenate([[0], np.cumsum(rpr * C)])
    table_row = base[q] + node_core * rpr[q] + (node_local - 128 * cw[q])
    bounds = [int(b) for b in base]
    return dict(starts=starts, counts=counts, NPC=NPC, gpc=gpc,
                node_core=node_core.astype(np.int64),
                node_local=node_local.astype(np.int64),
                table_row=table_row.astype(np.int64),
                cw=cw, bounds=bounds)


def build_direction_meta(gather_nodes, target_nodes, part, cfg):
    """Build per-core gather index / dstloc arrays and the uniform group
    structure for one edge direction.  Self-loops v->v are appended to the
    edge list so the epilogue needs no separate self term.  Slots within each
    (super, block, window) run are sorted by target position so each group's
    targets span a narrow band; per group a band (b0, wid in {64,128}) is
    chosen uniformly across cores and dstloc is stored band-relative.
    """
    N, C = cfg["N"], cfg["N_CORES"]
    SW, NBLK = cfg["SW"], cfg["NBLK"]
    NPC = part["NPC"]
    W = NPC // 128
    NS = (W + SW - 1) // SW

    loop = np.arange(N, dtype=np.int64)
    gather_nodes = np.concatenate([np.asarray(gather_nodes), loop])
    target_nodes = np.concatenate([np.asarray(target_nodes), loop])

    deg = np.bincount(target_nodes, minlength=N).astype(np.float64)

    bounds = part["bounds"]
    assert len(bounds) == NBLK + 1
    assert all(bounds[i + 1] - bounds[i] <= 32767 for i in range(NBLK))
    bounds_arr = np.array(bounds[1:-1])

    tr_g = part["table_row"][gather_nodes]
    t_core = part["node_core"][target_nodes]
    t_local = part["node_local"][target_nodes]
    lw = t_local // 128          # window
    dloc = t_local % 128         # position within window
    blk = np.searchsorted(bounds_arr, tr_g, side="right")
    idxv = tr_g - np.array(bounds[:-1])[blk]
    sup = lw // SW

    # per (core, s, b, w) counts -> uniform G
    keyW = (sup * NBLK + blk) * W + lw  # key within a core
    nkeys = NS * NBLK * W
    counts = np.zeros((C, nkeys), np.int64)
    for c in range(C):
        m = t_core == c
        counts[c] = np.bincount(keyW[m], minlength=nkeys)
    max_counts = counts.max(axis=0).reshape(NS, NBLK, W)

    G = np.ceil(max_counts / 128).astype(np.int64)  # groups per (s,b,w)
    # ensure every window has at least one group (psum must be written)
    for s in range(NS):
        w_lo, w_hi = s * SW, min((s + 1) * SW, W)
        for w in range(w_lo, w_hi):
            if G[s, :, w].sum() == 0:
                G[s, 0, w] = 1
        G[s, :, :w_lo] = 0
        G[s, :, w_hi:] = 0
    Gflat = G.reshape(nkeys)
    gmax = int(Gflat.max())

    # ---- per-core sorted runs (sorted by dloc within each key run) ----
    percore = []
    lo_all = np.full((nkeys, gmax), 128, np.int64)
    hi_all = np.full((nkeys, gmax), -1, np.int64)
    for c in range(C):
        m = t_core == c
        k = keyW[m]
        order = np.lexsort((dloc[m], k))
        ks = k[order]
        dvs = dloc[m][order]
        ivs = idxv[m][order]
        run_start = np.searchsorted(ks, np.arange(nkeys))
        rank = np.arange(len(ks)) - run_start[ks]
        percore.append((ks, dvs, ivs, rank))
        grp = np.minimum(rank // 128, gmax - 1)
        np.minimum.at(lo_all, (ks, grp), dvs)
        np.maximum.at(hi_all, (ks, grp), dvs)

    # 64-wide band, base in {0,64} (PSUM partition-base constraint: base 0/64
    # for 64-wide writes; base 32 allows only 32-wide)
    b0c = np.where(lo_all >= 64, 64, 0)
    fits = (hi_all < 0) | (hi_all <= b0c + 63)
    wid_all = np.where(fits, 64, 128)
    b0_all = np.where(fits & (hi_all >= 0), b0c, 0)

    # structure: per (s,b): window col bases, totals, group band records
    struct = []
    for s in range(NS):
        w_lo, w_hi = s * SW, min((s + 1) * SW, W)
        for b in range(NBLK):
            g_list = G[s, b, w_lo:w_hi]
            base = np.concatenate([[0], np.cumsum(g_list)])
            struct.append(dict(s=s, b=b, w_lo=w_lo, w_hi=w_hi,
                               g_list=g_list, g_base=base,
                               G=int(g_list.sum())))
    # global group/column offsets + per-group band records
    offG = 0
    off16 = 0
    n64 = n128 = 0
    for sb in struct:
        sb["offG"] = offG
        sb["off16"] = off16
        groups = []
        s, b = sb["s"], sb["b"]
        for i, w in enumerate(range(sb["w_lo"], sb["w_hi"])):
            key = (sb["s"] * NBLK + b) * W + w
            for g in range(int(sb["g_list"][i])):
                wid = int(wid_all[key, g])
                b0 = int(b0_all[key, g])
                if wid == 64:
                    groups.append((w, 64, n64, b0))
                    n64 += 1
                else:
                    groups.append((w, 128, n128, b0))
                    n128 += 1
        sb["groups"] = groups
        offG += sb["G"]
        off16 += sb["G"] * 8  # 128 slots / 16
    CG = offG
    CG64, CG128 = n64, n128
    Gmax = max((sb["G"] for sb in struct), default=1)

    # per-super class offsets (for dl tile loads)
    supers = []
    for si in range(NS):
        sbs = struct[si * NBLK:(si + 1) * NBLK]
        g64 = [g for sb in sbs for g in sb["groups"] if g[1] == 64]
        g128 = [g for sb in sbs for g in sb["groups"] if g[1] == 128]
        supers.append(dict(
            off64=g64[0][2] if g64 else 0, cnt64=len(g64),
            off128=g128[0][2] if g128 else 0, cnt128=len(g128)))

    # ---- per-edge slot assignment (per core) ----
    idx_all = np.zeros((C, 128, CG * 8), np.int16)
    dl64_all = np.full((C, 128, max(CG64, 1)), -1.0, BF16)
    dl128_all = np.full((C, 128, max(CG128, 1)), -1.0, BF16)
    # per (key, g): global slot base, class, class idx, b0
    slot_base = np.zeros((nkeys, gmax), np.int64)
    cls_arr = np.zeros((nkeys, gmax), np.int64)
    cidx_arr = np.zeros((nkeys, gmax), np.int64)
    for sb in struct:
        s, b = sb["s"], sb["b"]
        gi = 0
        for i, w in enumerate(range(sb["w_lo"], sb["w_hi"])):
            key = (s * NBLK + b) * W + w
            for g in range(int(sb["g_list"][i])):
                gslot = (sb["offG"] + sb["g_base"][i] + g) * 128
                slot_base[key, g] = gslot
                _, cls, cidx, _ = sb["groups"][gi]
                cls_arr[key, g] = cls
                cidx_arr[key, g] = cidx
                gi += 1

    for c in range(C):
        ks, dvs, ivs, rank = percore[c]
        grp = np.minimum(rank // 128, gmax - 1)
        slot = slot_base[ks, grp] + (rank - grp * 128)
        dshift = dvs - b0_all[ks, grp]
        # idx wrapped layout: slot j -> (j%16, j//16), replicated x8
        prow = slot % 16
        pcol = slot // 16
        tmp = np.zeros((16, CG * 8), np.int16)
        tmp[prow, pcol] = ivs.astype(np.int16)
        idx_all[c] = np.tile(tmp, (8, 1))
        cls_s = cls_arr[ks, grp]
        cidx_s = cidx_arr[ks, grp]
        srow = slot % 128
        m64 = cls_s == 64
        dl64_all[c, srow[m64], cidx_s[m64]] = dshift[m64].astype(BF16)
        m128 = ~m64
        dl128_all[c, srow[m128], cidx_s[m128]] = dshift[m128].astype(BF16)

    return dict(deg=deg, struct=struct, supers=supers, CG=CG,
                CG64=max(CG64, 1), CG128=max(CG128, 1), Gmax=Gmax, NS=NS, W=W,
                bounds=bounds, idx_all=idx_all,
                dl64_all=dl64_all, dl128_all=dl128_all)


def build_Mp(src, dst, batch, part, td_deg, bu_deg, cfg):
    """Folded layer-2 coefficients, local-row form, fp8 with MP_SCALE folded.

    out_graph[g] = (sum_s Mp[s,g] * h1[s]) @ W2 + n_g * b2, with the sum over
    LOCAL nodes s of each core (rows in node_local order)."""
    C, N, G = cfg["N_CORES"], cfg["N"], cfg["NUM_GRAPHS"]
    NPC = part["NPC"]
    nc_ = part["node_core"]
    nl = part["node_local"]
    batch = np.asarray(batch)

    dinv_td = (1.0 / np.sqrt(td_deg)).astype(np.float32)
    dinv_bu = (1.0 / np.sqrt(bu_deg)).astype(np.float32)

    M_td = np.zeros((C, NPC, G), np.float32)
    M_bu = np.zeros((C, NPC, G), np.float32)
    # td: value row src, target dst -> coeff dinv_td[dst] at (core(src), loc(src), g(dst))
    np.add.at(M_td, (nc_[src], nl[src], batch[dst]), dinv_td[dst])
    # bu: value row dst, target src -> coeff dinv_bu[src]
    np.add.at(M_bu, (nc_[dst], nl[dst], batch[src]), dinv_bu[src])
    # self-loop diagonals
    allv = np.arange(N)
    np.add.at(M_td, (nc_[allv], nl[allv], batch[allv]), dinv_td[allv])
    np.add.at(M_bu, (nc_[allv], nl[allv], batch[allv]), dinv_bu[allv])
    # fold the value-side dinv (from hn2 = dinv * (h1@W2)) into M
    f_td = np.zeros((C, NPC), np.float32)
    f_bu = np.zeros((C, NPC), np.float32)
    f_td[nc_[allv], nl[allv]] = dinv_td[allv]
    f_bu[nc_[allv], nl[allv]] = dinv_bu[allv]
    M_td *= f_td[:, :, None] * cfg["MP_SCALE"]
    M_bu *= f_bu[:, :, None] * cfg["MP_SCALE"]
    n_g = np.bincount(batch, minlength=G).astype(np.float32)
    return dict(Mp_td=M_td.astype(FP8), Mp_bu=M_bu.astype(FP8), n_g=n_g,
                dinv_td=dinv_td, dinv_bu=dinv_bu)


def rebalance_partition(part, src, dst, cfg):
    """Second balancing pass: reassign each core's nodes to windows (within
    their chunk's window range, so source-chunk memberships stay valid) so
    that per-(window, direction, source-chunk) edge counts are as uniform as
    possible across windows AND cores.  This shrinks the ceil-128 padding of
    the gather groups (the dominant SWDGE packet cost)."""
    N, C, NBLK = cfg["N"], cfg["N_CORES"], cfg["NBLK"]
    W = part["NPC"] // 128
    cw = part["cw"]
    chunk_of_w = np.searchsorted(cw[1:], np.arange(W), side="right")
    node_chunk = chunk_of_w[np.minimum(part["node_local"] // 128, W - 1)]
    deg8 = np.zeros((N, 2 * NBLK), np.int32)
    np.add.at(deg8, (dst, node_chunk[src]), 1)            # td in-edges by src chunk
    np.add.at(deg8, (src, NBLK + node_chunk[dst]), 1)     # bu
    allv = np.arange(N)
    np.add.at(deg8, (allv, node_chunk[allv]), 1)          # self loops
    np.add.at(deg8, (allv, NBLK + node_chunk[allv]), 1)
    node_local = part["node_local"].copy()
    for c in range(C):
        lo, hi = part["starts"][c], part["starts"][c + 1]
        nl = node_local[lo:hi]
        for q in range(NBLK):
            wlo, whi = int(cw[q]), int(cw[q + 1])
            idxs = np.nonzero((nl // 128 >= wlo) & (nl // 128 < whi))[0]
            Wq = whi - wlo
            dv = deg8[lo + idxs].astype(np.float64)
            # quantized per-window targets: distribute ceil-128 group levels
            # so each (window, comp) aims just under a multiple of 128, with
            # the +1-level windows staggered across components
            totmax = np.zeros(2 * NBLK)
            for cc in range(C):
                llo, lhi = part["starts"][cc], part["starts"][cc + 1]
                nlc = part["node_local"][llo:lhi]
                mq = (nlc // 128 >= wlo) & (nlc // 128 < whi)
                totmax = np.maximum(totmax, deg8[llo:lhi][mq].sum(0))
            T = np.zeros((Wq, 2 * NBLK))
            for b in range(2 * NBLK):
                gb = max(Wq, int(np.ceil(totmax[b] * 1.03 / 128)))
                k, rem = gb // Wq, gb % Wq
                T[:, b] = 128 * k
                off = (b * Wq) // (2 * NBLK)
                T[(off + np.arange(rem)) % Wq, b] += 128
            order = np.argsort(-dv.sum(1), kind="stable")
            load = np.zeros((Wq, 2 * NBLK))
            room = np.full(Wq, 128, np.int64)
            assign = np.empty(len(idxs), np.int64)
            for j in order:
                nxt = load + dv[j][None, :] - T
                over = np.maximum(nxt, 0).sum(axis=1)
                score = over * 1e6 + nxt.max(axis=1)
                score[room <= 0] = np.inf
                w = int(np.argmin(score))
                assign[j] = w
                load[w] += dv[j]
                room[w] -= 1
            slot_in = np.zeros(Wq, np.int64)
            for j in range(len(idxs)):
                w = assign[j]
                nl[idxs[j]] = (wlo + w) * 128 + slot_in[w]
                slot_in[w] += 1
        node_local[lo:hi] = nl
    q = chunk_of_w[np.minimum(node_local // 128, W - 1)]
    rpr = 128 * np.diff(cw)
    base = np.concatenate([[0], np.cumsum(rpr * C)])
    table_row = base[q] + part["node_core"] * rpr[q] + (node_local - 128 * cw[q])
    part = dict(part)
    part["node_local"] = node_local.astype(np.int64)
    part["table_row"] = table_row.astype(np.int64)
    return part


def build_all_inputs(x, edge_index, batch, Ws, bs, cfg):
    """Produce per-core in_maps plus structural metadata."""
    C = cfg["N_CORES"]
    N = cfg["N"]
    src = np.asarray(edge_index[0])
    dst = np.asarray(edge_index[1])
    part = build_partition(batch, cfg,
                           deg_td=np.bincount(dst, minlength=N),
                           deg_bu=np.bincount(src, minlength=N))
    part = rebalance_partition(part, src, dst, cfg)
    NPC = part["NPC"]
    W = NPC // 128
    R = C * NPC
    RW = R // 128

    td = build_direction_meta(src, dst, part, cfg)   # gather src row, scatter to dst
    bu = build_direction_meta(dst, src, part, cfg)   # reversed
    mp = build_Mp(src, dst, batch, part, td["deg"], bu["deg"], cfg)

    PIECE_G = cfg["PIECE_G"]
    iota128 = np.tile(np.arange(128, dtype=np.float32), PIECE_G)[None, :] \
        .repeat(128, 0).astype(BF16)
    iota64 = np.tile(np.arange(64, dtype=np.float32), PIECE_G)[None, :] \
        .repeat(128, 0).astype(BF16)

    # global fp8 x in table-row order + global per-row dinv (both directions
    # interleaved) used to scale the A1 psum before the fp8 table store
    tr = part["table_row"]
    xTg = np.zeros((256, R), FP8)
    xTg[:, tr] = np.asarray(x).T.astype(FP8)
    dinvg = np.ones((R, 2), np.float32)
    dinvg[tr, 0] = mp["dinv_td"]
    dinvg[tr, 1] = mp["dinv_bu"]
    dinvg = np.ascontiguousarray(
        dinvg.reshape(RW, 128, 2).transpose(1, 0, 2).reshape(128, RW * 2))
    W1cat = np.concatenate([Ws[0], Ws[2]], axis=1).astype(FP8)  # [256, 256]

    ngb2 = np.concatenate([np.outer(mp["n_g"], bs[1]),
                           np.outer(mp["n_g"], bs[3])], axis=1).astype(np.float32)

    # per-core tensors
    in_maps = []
    for c in range(C):
        lo, hi = part["starts"][c], part["starts"][c + 1]
        li = part["node_local"][lo:hi]
        dloc = np.ones((128, W, 2), np.float32)
        dloc[li % 128, li // 128, 0] = mp["dinv_td"][lo:hi]
        dloc[li % 128, li // 128, 1] = mp["dinv_bu"][lo:hi]
        im = dict(
            xTg=xTg,
            dinvg=dinvg,
            W1cat=W1cat,
            dinv_loc=dloc.reshape(128, W * 2),
            iota64=iota64, iota128=iota128,
            Mp_td=mp["Mp_td"][c], Mp_bu=mp["Mp_bu"][c],
            idx_td=td["idx_all"][c], idx_bu=bu["idx_all"][c],
            dl64_td=td["dl64_all"][c], dl128_td=td["dl128_all"][c],
            dl64_bu=bu["dl64_all"][c], dl128_bu=bu["dl128_all"][c],
            W_td2=(Ws[1] / cfg["MP_SCALE"]).astype(BF16),
            W_bu2=(Ws[3] / cfg["MP_SCALE"]).astype(BF16),
            b_td1=np.tile(bs[0][None, :], (128, 1)).astype(np.float32),
            b_bu1=np.tile(bs[2][None, :], (128, 1)).astype(np.float32),
        )
        in_maps.append(im)
    meta = dict(part=part, td=td, bu=bu, NPC=NPC, W=W, cfg=cfg,
                R=R, RW=RW, ngb2=ngb2)
    return in_maps, meta


# =====================================================================
# Bass program
# =====================================================================

def build_bass(meta):
    import concourse.bacc as bacc
    import concourse.mybir as mybir
    import concourse.tile as tile

    cfg = meta["cfg"]
    C = cfg["N_CORES"]
    NPC, W = meta["NPC"], meta["W"]
    IN, HID = cfg["IN_FEATS"], cfg["HIDDEN"]
    NBLK = cfg["NBLK"]
    NG = cfg["NUM_GRAPHS"]
    PIECE_G = cfg["PIECE_G"]
    R, RW = meta["R"], meta["RW"]
    f32, bf16, i16 = mybir.dt.float32, mybir.dt.bfloat16, mybir.dt.int16
    fp8 = mybir.dt.float8e4
    DR = mybir.MatmulPerfMode.DoubleRow

    nc = bacc.Bacc("TRN2", target_bir_lowering=False, debug=False, num_devices=C,
                   num_swdge_queues=4)

    # ---- I/O ----
    ten = {}
    def inp(name, shape, dt):
        ten[name] = nc.dram_tensor(name, shape, dt, kind="ExternalInput")
        return ten[name]

    inp("dinv_loc", [128, W * 2], f32)
    inp("iota64", [128, PIECE_G * 64], bf16)
    inp("iota128", [128, PIECE_G * 128], bf16)
    inp("xTg", [IN, R], fp8)
    inp("dinvg", [128, RW * 2], f32)
    inp("W1cat", [IN, 2 * HID], fp8)
    for d in ("td", "bu"):
        m = meta[d]
        inp(f"idx_{d}", [128, m["CG"] * 8], i16)
        inp(f"dl64_{d}", [128, m["CG64"]], bf16)
        inp(f"dl128_{d}", [128, m["CG128"]], bf16)
        inp(f"Mp_{d}", [NPC, NG], fp8)
        inp(f"W_{d}2", [HID, HID], bf16)
        inp(f"b_{d}1", [128, HID], f32)
    out_t = nc.dram_tensor("out", [NG, 2 * HID], f32, kind="ExternalOutput")

    # internal DRAM: per-block packed hn tables [rows_b, 256] fp8
    bounds = meta["td"]["bounds"]
    table = {}
    for b in range(NBLK):
        table[b] = nc.dram_tensor(
            f"table{b}", [bounds[b + 1] - bounds[b], 2 * HID], fp8,
            kind="Internal")

    from contextlib import ExitStack
    with tile.TileContext(nc) as tc, ExitStack() as stack:
        def pool(name, bufs, space="SBUF"):
            return stack.enter_context(tc.tile_pool(name=name, bufs=bufs, space=space))

        const = pool("const", 1)
        xt_p = pool("xt", 8)
        hnb_p = pool("hnb", 4)
        idx_p = pool("idx", 6)
        dl_p = pool("dl", 6)
        gat_p = pool("gat", 10)              # gathered edge tiles (fp8)
        oh_p = pool("oh", 5)                 # one-hot tiles (fp8)
        mp_p = pool("mp", 3)                 # Mp super tiles
        win_p = pool("win", 4, "PSUM")       # window psum, 4 windows/bank
        pps_p = pool("pps", 3, "PSUM")       # A1 psum pairs / P-partial halves
        hps_p = pool("hps", 1, "PSUM")       # final projection psum
        epi_p = pool("epi", 6)               # epilogue sbuf tiles
        h1_p = pool("h1", 4)
        accs = pool("accs", 1)               # P accumulator (SBUF, f32)

        # ---- constants in SBUF ----
        iota_t = {}
        for wd, cols in ((64, PIECE_G * 64), (128, PIECE_G * 128)):
            t = const.tile([128, cols], bf16, tag=f"iota{wd}")
            nc.sync.dma_start(t[:], ten[f"iota{wd}"][:])
            iota_t[wd] = t
        W1c = []
        for kk in range(IN // 128):
            t = const.tile([128, 2 * HID], fp8, tag=f"W1c{kk}", name=f"W1c{kk}")
            nc.sync.dma_start(t[:], ten["W1cat"][kk * 128:(kk + 1) * 128, :])
            W1c.append(t)
        dinvg_t = const.tile([128, RW * 2], f32, tag="dinvg")
        nc.sync.dma_start(dinvg_t[:], ten["dinvg"][:])
        W2t, bt = {}, {}
        for d in ("td", "bu"):
            t = const.tile([128, HID], bf16, tag=f"W2_{d}", name=f"W2_{d}")
            nc.sync.dma_start(t[:], ten[f"W_{d}2"][:])
            W2t[d] = t
            t = const.tile([128, HID], f32, tag=f"b_{d}1", name=f"bt_{d}1")
            nc.sync.dma_start(t[:], ten[f"b_{d}1"][:])
            bt[d] = t
        dinvl_t = const.tile([128, W * 2], f32, tag="dinvl")
        nc.sync.dma_start(dinvl_t[:], ten["dinv_loc"][:])
        zrow = const.tile([1, 512], bf16, tag="zrow")
        nc.gpsimd.memset(zrow[:], 0.0)

        # P accumulator [128f, td 1024g | bu 1024g] f32
        acc = accs.tile([128, 2 * NG], f32, tag="acc", name="acc")
        nc.gpsimd.memset(acc[:], 0.0)

        cw = meta["part"]["cw"]
        chunk_base_oct = [int(cw[q]) for q in range(NBLK + 1)]  # x8 windows = octets

        # ---- A1 (replicated): hn = dinv * (x @ W1), both directions packed,
        # written chunk-major to the fp8 tables ----
        nK = IN // 128

        def emit_a1_octet(oct_i):
            q = 0
            while chunk_base_oct[q + 1] * 8 <= oct_i * 8:
                q += 1
            xts = []
            for kk in range(nK):
                t = xt_p.tile([128, 8 * 128], fp8, tag="xt", name=f"xa_{oct_i}_{kk}")
                nc.sync.dma_start(
                    t[:], ten["xTg"][kk * 128:(kk + 1) * 128,
                                     oct_i * 1024:(oct_i + 1) * 1024])
                xts.append(t)
            hnb = hnb_p.tile([128, 8, 2 * HID], fp8, tag="hnb", name=f"hnb_{oct_i}")
            for j0 in range(0, 8, 2):
                hps = pps_p.tile([128, 2, 2 * HID], f32, tag="pps")
                for j in (j0, j0 + 1):
                    for kk in range(nK):
                        nc.tensor.matmul(hps[:, j - j0, :],
                                         xts[kk][:, j * 128:(j + 1) * 128],
                                         W1c[kk][:], start=(kk == 0),
                                         stop=(kk == nK - 1),
                                         skip_group_check=True)
                c0 = (oct_i * 8 + j0) * 2
                nc.vector.tensor_tensor(
                    out=hnb[:, j0:j0 + 2, :].rearrange("p j (t f) -> p (j t) f", f=HID),
                    in0=hps[:].rearrange("p j (t f) -> p (j t) f", f=HID),
                    in1=dinvg_t[:, c0:c0 + 4]
                    .rearrange("p (x o) -> p x o", o=1)
                    .to_broadcast([128, 4, HID]),
                    op=mybir.AluOpType.mult)
            rowb = oct_i * 1024 - bounds[q]
            nc.scalar.dma_start(
                table[q][rowb:rowb + 1024, :].rearrange("(j p) f -> p j f", p=128),
                hnb[:])

        # ---- edge-phase per-super loads (idx/dl on sync pre-A1 for the first
        # supers, then on the gpsimd queue so they never sit behind A1 DMA) ----
        sup_tiles = {}

        def emit_super_loads(d, si, eng):
            m = meta[d]
            structs = m["struct"]
            sb0 = structs[si * NBLK]
            supG = sum(x["G"] for x in structs[si * NBLK:(si + 1) * NBLK])
            rec = m["supers"][si]
            ts = {}
            if supG > 0:
                t = idx_p.tile([128, supG * 8], i16, tag="idx")
                eng.dma_start(t[:], ten[f"idx_{d}"]
                              [:, sb0["off16"]:sb0["off16"] + supG * 8])
                ts["idx"] = t
            for wd in (64, 128):
                cnt = rec[f"cnt{wd}"]
                if cnt > 0:
                    t = dl_p.tile([128, cnt], bf16, tag=f"dl{wd}")
                    off = rec[f"off{wd}"]
                    eng.dma_start(t[:], ten[f"dl{wd}_{d}"][:, off:off + cnt])
                    ts[f"dl{wd}"] = t
            sup_tiles[(d, si)] = ts

        PREF = 3
        for si in range(min(PREF, meta["td"]["NS"])):
            for d in ("td", "bu"):
                emit_super_loads(d, si, nc.sync)

        for oct_i in range(RW // 8):
            emit_a1_octet(oct_i)

        # ---- edge phase (layer-1 aggregation + fused pooled layer-2) ----
        qn = [0]

        def epilogue(d, w, pt, mpt, wi, pps, w_lo, w_hi):
            di = 0 if d == "td" else 1
            o1 = epi_p.tile([128, HID], f32, tag="o1")
            nc.vector.scalar_tensor_tensor(
                out=o1[:], in0=pt[:], scalar=dinvl_t[:, w * 2 + di:w * 2 + di + 1],
                in1=bt[d][:],
                op0=mybir.AluOpType.mult, op1=mybir.AluOpType.add)
            h1 = h1_p.tile([128, HID], fp8, tag="h1")
            nc.scalar.activation(h1[:], o1[:], mybir.ActivationFunctionType.Relu)
            for h in range(2):
                nc.tensor.matmul(pps[h][:], h1[:], mpt[:, wi, h * 512:(h + 1) * 512],
                                 start=(w == w_lo), stop=(w == w_hi - 1),
                                 skip_group_check=True)

        def edge_phase(d):
            m = meta[d]
            doff = 0 if d == "td" else HID
            structs = m["struct"]
            supers = m["supers"]
            last_mm = {}
            for sbi, sb in enumerate(structs):
                gi = 0
                for i, w in enumerate(range(sb["w_lo"], sb["w_hi"])):
                    for g in range(int(sb["g_list"][i])):
                        if sb["g_list"][i] > 0:
                            last_mm[w] = (sbi, int(sb["g_base"][i]) + g)
            quad_tiles = {}
            def win_q(w):
                q = w // 4
                if q not in quad_tiles:
                    qt = win_p.tile([128, 512], f32, tag="win",
                                    name=f"win_{d}_{q}")
                    nc.tensor.matmul(qt[:], zrow[0:1, 0:128], zrow[0:1, 0:512],
                                     start=True, stop=False, skip_group_check=True)
                    quad_tiles[q] = qt
                return quad_tiles[q]
            it_sup = None
            dlt = {}
            sup_rec = None
            sup_off16 = 0
            for sbi, sb in enumerate(structs):
                si = sbi // NBLK
                if sbi % NBLK == 0:
                    sup_off16 = sb["off16"]
                    sup_rec = supers[si]
                    if (d, si) not in sup_tiles:
                        emit_super_loads(d, si, nc.scalar)
                    ts = sup_tiles[(d, si)]
                    it_sup = ts.get("idx")
                    dlt = {64: ts.get("dl64"), 128: ts.get("dl128")}
                    # Mp tile for the whole super (both layer-2 halves)
                    nsw = sb["w_hi"] - sb["w_lo"]
                    mpt = mp_p.tile([128, nsw, NG], fp8, tag="mp",
                                    name=f"mp_{d}_{si}")
                    nc.scalar.dma_start(
                        mpt[:], ten[f"Mp_{d}"][sb["w_lo"] * 128:sb["w_hi"] * 128, :]
                        .rearrange("(j p) g -> p j g", p=128))
                    sup_mpt = mpt
                G = sb["G"]
                if G == 0:
                    continue
                r16 = sb["off16"] - sup_off16
                # flat (window, group-record) list for this (s,b)
                wg = []
                gi = 0
                for i, w in enumerate(range(sb["w_lo"], sb["w_hi"])):
                    for g in range(int(sb["g_list"][i])):
                        wg.append((w, int(sb["g_base"][i]) + g) + sb["groups"][gi])
                        gi += 1
                # split into pieces of <= PIECE_G groups
                npiece = (G + PIECE_G - 1) // PIECE_G
                for pi in range(npiece):
                    g0 = pi * G // npiece
                    g1 = (pi + 1) * G // npiece
                    pg = g1 - g0
                    gt = gat_p.tile([128, pg, 2 * HID], fp8, tag="gat")
                    qn[0] += 1
                    nc.gpsimd.dma_gather(gt[:], table[sb["b"]][:],
                                         it_sup[:, r16 + g0 * 8:r16 + g1 * 8],
                                         num_idxs=pg * 128,
                                         num_idxs_reg=pg * 128, elem_size=2 * HID,
                                         single_packet=False, queue_num=qn[0] % 4)
                    # one-hots per band class
                    ohs = {}
                    cls_lo = {}
                    for wd in (64, 128):
                        recs = [r for r in wg[g0:g1] if r[3] == wd]
                        if not recs:
                            continue
                        c_lo = recs[0][4]
                        ncl = len(recs)
                        cls_lo[wd] = c_lo
                        oh = oh_p.tile([128, PIECE_G * wd], fp8, tag=f"oh{wd}")
                        ohs[wd] = oh
                        s_off = sup_rec[f"off{wd}"]
                        nc.vector.tensor_tensor(
                            out=oh[:, :ncl * wd].rearrange("p (g f) -> p g f", f=wd),
                            in0=dlt[wd][:, c_lo - s_off:c_lo - s_off + ncl]
                            .rearrange("p (g o) -> p g o", o=1)
                            .to_broadcast([128, ncl, wd]),
                            in1=iota_t[wd][:, :ncl * wd]
                            .rearrange("p (g f) -> p g f", f=wd),
                            op=mybir.AluOpType.is_equal)
                    for (w, g, _w2, wd, cidx, b0) in wg[g0:g1]:
                        qt = win_q(w)
                        ci = cidx - cls_lo[wd]
                        nc.tensor.matmul(
                            qt[b0:b0 + wd, (w % 4) * 128:(w % 4 + 1) * 128],
                            ohs[wd][:, ci * wd:(ci + 1) * wd],
                            gt[:, g - g0, doff:doff + HID],
                            start=False, stop=(last_mm[w] == (sbi, g)),
                            skip_group_check=True)
                # epilogues for completed supers: after last block of super
                if sb["b"] == NBLK - 1:
                    pps = [pps_p.tile([128, 512], f32, tag="pps",
                                      name=f"pps{h}_{d}_{sb['s']}")
                           for h in range(2)]
                    for wi, w in enumerate(range(sb["w_lo"], sb["w_hi"])):
                        pt = win_q(w)[:, (w % 4) * 128:(w % 4 + 1) * 128]
                        epilogue(d, w, pt, sup_mpt, wi, pps, sb["w_lo"], sb["w_hi"])
                    # fold the super's P-partial into the SBUF accumulator
                    aoff = 0 if d == "td" else NG
                    for h in range(2):
                        nc.vector.tensor_tensor(
                            out=acc[:, aoff + h * 512:aoff + (h + 1) * 512],
                            in0=acc[:, aoff + h * 512:aoff + (h + 1) * 512],
                            in1=pps[h][:], op=mybir.AluOpType.add)
                    quad_tiles.clear()
                    yield sb["w_hi"]
                else:
                    yield None

        def run_layer():
            gens = {"td": edge_phase("td"), "bu": edge_phase("bu")}
            done = {"td": False, "bu": False}
            while not all(done.values()):
                for d in ("td", "bu"):
                    if done[d]:
                        continue
                    try:
                        next(gens[d])
                    except StopIteration:
                        done[d] = True

        run_layer()

        # ---- final projection: out_partial[g] = P^T @ W2 (host sums partials) ----
        for d, (aoff, ooff) in (("td", (0, 0)), ("bu", (NG, HID))):
            for gc in range(NG // 128):
                pb = epi_p.tile([128, 128], bf16, tag="pb")
                nc.vector.tensor_copy(pb[:], acc[:, aoff + gc * 128:aoff + (gc + 1) * 128])
                fps = hps_p.tile([128, 4, HID], f32, tag="hps")
                nc.tensor.matmul(fps[:, 0, :], pb[:], W2t[d][:], start=True, stop=True,
                                 skip_group_check=True)
                ob = epi_p.tile([128, HID], f32, tag="ob")
                nc.vector.tensor_copy(ob[:], fps[:, 0, :])
                nc.sync.dma_start(out_t[gc * 128:(gc + 1) * 128, ooff:ooff + HID], ob[:])

    nc.compile()
    return nc


# =====================================================================
# Entry point
# =====================================================================

def _run(inputs, cfg, trace=False):
    from concourse import bass_utils
    x = np.asarray(inputs["x"], np.float32)
    edge_index = np.asarray(inputs["edge_index"])
    batch = np.asarray(inputs["batch"])
    Ws = [np.asarray(inputs[k], np.float32) for k in ("W_td1", "W_td2", "W_bu1", "W_bu2")]
    bs = [np.asarray(inputs[k], np.float32) for k in ("b_td1", "b_td2", "b_bu1", "b_bu2")]
    in_maps, meta = build_all_inputs(x, edge_index, batch, Ws, bs, cfg)
    nc = build_bass(meta)
    res = bass_utils.run_bass_kernel_spmd(
        nc, in_maps, core_ids=list(range(cfg["N_CORES"])), trace=trace)
    out = sum(res.results[c]["out"].astype(np.float64) for c in range(cfg["N_CORES"]))
    out = out + meta["ngb2"].astype(np.float64)
    return out.astype(np.float32), res


def kernel(**inputs):
    out, _ = _run(inputs, FULL_CFG, trace=False)
    return out
